# revision 13
# baseline (speedup 1.0000x reference)
"""Trainium2 Bass kernel for nn_BinaryBiaffine2 (biaffine dependency scorer).

Math (per batch b):
    h_dep  = leaky_relu(hidden @ W_dep  + b_dep)             [L, 500]
    h_head = leaky_relu(hidden @ W_head + b_head)            [L, 500]
    dep    = h_dep  @ Wc[:500]                               [L, 2]
    head   = h_head @ Wc[500:]                               [L, 2]
    out[i, j, c] = dep[i, c] + head[j, c] + bc[c]            [L, L, 2]

Sharding: data-parallel over batch, 2 batches per core on 8 cores.

Per-core strategy (v3):
  - hidden loaded natural ([tok, d]), PE-transposed 128x128 into hT tiles
    [d, tok] stored bf16 (identity is bf16 so transposes run at 1 cyc/row).
  - weights bf16 (halves weight HBM traffic; matmuls full-rate at any N).
  - MLP in [m, tok] layout; leaky(x+b) = relu(0.99x+0.99b) + 0.01*(x+b)
    with the linear term in bf16 so the DVE add runs in 2x mode.
  - head scores broadcast across partitions via matmul with a
    partition-replicated Wc column as stationary (+bc folded in the copy).
  - dep branch runs in token QUARTERS (256 tok) so assembly+stores stream
    at fine granularity; all DMAs (loads+stores) issue from the SP queue
    in readiness order (single shared DMA device in hw).
  - batch 1 (last) computes its head branch in halves: out[i, j in h0]
    tiles store while head-h1 is still being computed, shrinking the
    end-of-kernel store tail.
"""

import os
import sys

if "/opt/trn_rl_repo" not in sys.path:
    sys.path.insert(0, "/opt/trn_rl_repo")

import numpy as np

B, L, D = 16, 1024, 1024
MLP = 500
MLP_PAD = 512
NEG_SLOPE = 0.01
N_CORES = 8
B_PER_CORE = B // N_CORES
P = 128
N_MT = MLP_PAD // P  # 4 m-tiles of 128
N_KO = D // P        # 8 d-slices of 128
N_TSUB = L // P      # 8 token subtiles per batch
NQ = 4               # dep-branch quarters per batch (256 tok each)
QTOK = L // NQ

_CACHE = {}


def _build_nc():
    import concourse.tile as tile
    from concourse import bacc, mybir
    from concourse.bass import ts
    from contextlib import ExitStack

    f32 = mybir.dt.float32
    f32r = mybir.dt.float32r
    bf16 = mybir.dt.bfloat16
    Relu = mybir.ActivationFunctionType.Relu
    Identity = mybir.ActivationFunctionType.Identity
    Add = mybir.AluOpType.add
    Mult = mybir.AluOpType.mult

    nc = bacc.Bacc()

    hidden = nc.dram_tensor("hidden", [B_PER_CORE, L, D], f32r, kind="ExternalInput")
    w_dep_d = nc.dram_tensor("w_dep", [D, MLP_PAD], bf16, kind="ExternalInput")
    w_head_d = nc.dram_tensor("w_head", [D, MLP_PAD], bf16, kind="ExternalInput")
    # bias tiles: columns (2*mt, 2*mt+1) = (0.99*b, b) for m-tile mt
    b_dep_d = nc.dram_tensor("b_dep_t", [P, 2 * N_MT], f32, kind="ExternalInput")
    b_head_d = nc.dram_tensor("b_head_t", [P, 2 * N_MT], f32, kind="ExternalInput")
    wc_dep_d = nc.dram_tensor("wc_dep_t", [P, N_MT, 2], bf16, kind="ExternalInput")
    wc_head_d = nc.dram_tensor("wc_head_bc", [P, 2, N_MT, P], bf16, kind="ExternalInput")
    bc_d = nc.dram_tensor("bc_bc", [P, 2], f32, kind="ExternalInput")
    ident_d = nc.dram_tensor("ident", [P, P], f32r, kind="ExternalInput")
    out_d = nc.dram_tensor("out", [B_PER_CORE, L, L, 2], f32, kind="ExternalOutput")

    with tile.TileContext(nc) as tc:
        with ExitStack() as ctx:
            const = ctx.enter_context(tc.tile_pool(name="const", bufs=1))
            hnat_p = ctx.enter_context(tc.tile_pool(name="hnat", bufs=4))
            hT_p = ctx.enter_context(tc.tile_pool(name="hT", bufs=24))
            lh_p = ctx.enter_context(tc.tile_pool(name="lh", bufs=5))
            lhd_p = ctx.enter_context(tc.tile_pool(name="lhd", bufs=8))
            tmp_p = ctx.enter_context(tc.tile_pool(name="tmp", bufs=3))
            dept_p = ctx.enter_context(tc.tile_pool(name="dept", bufs=2))
            depsc_p = ctx.enter_context(tc.tile_pool(name="depsc", bufs=2))
            hbc_p = ctx.enter_context(tc.tile_pool(name="hbc", bufs=4))
            out_p = ctx.enter_context(tc.tile_pool(name="outp", bufs=5))
            outh_p = ctx.enter_context(tc.tile_pool(name="outh", bufs=6))
            tr_ps = ctx.enter_context(tc.tile_pool(name="trps", bufs=4, space="PSUM"))
            h_ps = ctx.enter_context(tc.tile_pool(name="hps", bufs=4, space="PSUM"))

            # ---- constants: tiny ones first, then batch-0 hidden, then
            # weights; all on the SP (sync) HWDGE queue which carries every
            # DMA so the shared DMA device runs them in readiness order.
            ident_sb = const.tile([P, P], f32r)
            nc.sync.dma_start(ident_sb, ident_d[:, :])

            hid_r = hidden[:, :, :]

            def load_batch(b, fine):
                h_nats = []
                for tp in range(N_TSUB // 2):
                    h_nat = hnat_p.tile([P, 2, D], f32r, name="h_nat")
                    if fine:
                        for s in range(2):
                            nc.sync.dma_start(
                                h_nat[:, s],
                                hid_r[b, ts(2 * tp + s, P), :],
                            )
                    else:
                        nc.sync.dma_start(
                            h_nat,
                            hid_r[b, ts(tp, 2 * P), :].rearrange(
                                "(s p) d -> p s d", p=P
                            ),
                        )
                    h_nats.append(h_nat)
                return h_nats

            # batch-0 half 0 tokens first, then w_head (so the head MLP can
            # start on half 0 while half 1 + w_dep are still loading)
            w_sb = {}
            loaded = []

            def load_b0_half(half):
                for tp in range(2 * half, 2 * half + 2):
                    h_nat = hnat_p.tile([P, 2, D], f32r, name="h_nat")
                    for s in range(2):
                        nc.sync.dma_start(
                            h_nat[:, s], hid_r[0, ts(2 * tp + s, P), :]
                        )
                    loaded.append(h_nat)

            load_b0_half(0)
            b_sb = {}
            b_dep_sb = const.tile([P, 2 * N_MT], f32)
            nc.sync.dma_start(b_dep_sb, b_dep_d[:, :])
            b_head_sb = const.tile([P, 2 * N_MT], f32)
            nc.sync.dma_start(b_head_sb, b_head_d[:, :])
            b_sb["dep"], b_sb["head"] = b_dep_sb, b_head_sb
            wc_dep_sb = const.tile([P, N_MT, 2], bf16)
            nc.sync.dma_start(wc_dep_sb, wc_dep_d[:, :, :])
            bc_sb = const.tile([P, 2], f32)
            nc.sync.dma_start(bc_sb, bc_d[:, :])
            w_head_sb = const.tile([P, N_KO, MLP_PAD], bf16)
            nc.sync.dma_start(
                w_head_sb, w_head_d[:, :].rearrange("(ko p) m -> p ko m", p=P)
            )
            load_b0_half(1)
            w_dep_sb = const.tile([P, N_KO, MLP_PAD], bf16)
            nc.sync.dma_start(
                w_dep_sb, w_dep_d[:, :].rearrange("(ko p) m -> p ko m", p=P)
            )
            w_sb["dep"], w_sb["head"] = w_dep_sb, w_head_sb
            wc_head_sb = const.tile([P, 2, N_MT, P], bf16)
            nc.sync.dma_start(wc_head_sb, wc_head_d[:, :, :, :])

            # rotating engine assignment for elementwise output/copy units
            def eng_ring(seq):
                i = [0]

                def nxt():
                    e = seq[i[0] % len(seq)]
                    i[0] += 1
                    return e

                return nxt

            def transposes(h_nats, halves=(0, 1), hTs=None):
                """PE-transpose a batch into hT tiles [d=128, tok=512] bf16."""
                if hTs is None:
                    hTs = {}
                cp = eng_ring([nc.vector, nc.vector, nc.scalar])
                for half in halves:
                    for ko in range(N_KO):
                        ptr = tr_ps.tile([P, 512], f32r, name="ptr")
                        for q in range(4):
                            tsub = half * 4 + q
                            nc.tensor.matmul(
                                ptr[:, ts(q, P)],
                                lhsT=h_nats[tsub // 2][:, tsub % 2, ts(ko, P)],
                                rhs=ident_sb,
                                is_transpose=True,
                                start=True,
                                stop=True,
                            )
                        hT = hT_p.tile([P, 512], bf16, name="hT")
                        e = cp()
                        if e is nc.scalar:
                            e.activation(hT, ptr, Identity)
                        else:
                            e.tensor_copy(hT, ptr)
                        hTs[half, ko] = hT
                return hTs

            def leaky(dst, ps, br, mt):
                """dst(bf16) = leaky_relu(ps + b) via relu(0.99x+0.99b) +
                0.01(x+b); the bf16 add runs in DVE 2x mode."""
                lt = tmp_p.tile(list(dst.shape), bf16, name="lt")
                nc.scalar.activation(
                    dst, ps, Relu,
                    bias=b_sb[br][:, 2 * mt : 2 * mt + 1],
                    scale=1.0 - NEG_SLOPE,
                )
                nc.vector.tensor_scalar(
                    lt, ps,
                    b_sb[br][:, 2 * mt + 1 : 2 * mt + 2], NEG_SLOPE,
                    Add, Mult,
                )
                nc.vector.tensor_add(dst, dst, lt)

            def head_mlp(hTs, halves, tiles=None):
                """Head branch for the given token halves -> lh tiles
                [m=128, L] bf16 keyed by mt (columns half*512.. filled).
                Emits all of half A before half B so the PE can run as soon
                as half A's hT tiles exist."""
                if tiles is None:
                    tiles = {mt: lh_p.tile([P, L], bf16, name="lh")
                             for mt in range(N_MT)}
                for half in halves:
                    for mt in range(N_MT):
                        ps = h_ps.tile([P, 512], f32, name="hps")
                        for ko in range(N_KO):
                            nc.tensor.matmul(
                                ps,
                                lhsT=w_sb["head"][:, ko, ts(mt, P)],
                                rhs=hTs[half, ko],
                                start=(ko == 0),
                                stop=(ko == N_KO - 1),
                            )
                        leaky(tiles[mt][:, ts(half, 512)], ps, "head", mt)
                return tiles

            def head_bc_phase(lh_tiles, halves, hb_tiles):
                """Head scores, partition-broadcast, +bc folded; writes the
                [:, half*512:...] columns of hb_tiles[c]."""
                for half in halves:
                    for c in range(2):
                        pbc = h_ps.tile([P, 512], f32, name="hps")
                        for mt in range(N_MT):
                            nc.tensor.matmul(
                                pbc,
                                lhsT=wc_head_sb[:, c, mt, :],
                                rhs=lh_tiles[mt][:, ts(half, 512)],
                                start=(mt == 0),
                                stop=(mt == N_MT - 1),
                            )
                        nc.scalar.activation(
                            hb_tiles[c][:, ts(half, 512)],
                            pbc,
                            Identity,
                            bias=bc_sb[:, c : c + 1],
                        )

            def dep_quarter_mm(hTs, q):
                """Dep-branch MLP for token quarter q -> lh tiles [m, 256]."""
                half, qc = q // 2, q % 2
                tiles = {}
                for mt in range(N_MT):
                    psd = h_ps.tile([P, QTOK], f32, name="hps",
                                    padded_shape=[P, 512])
                    for ko in range(N_KO):
                        nc.tensor.matmul(
                            psd,
                            lhsT=w_sb["dep"][:, ko, ts(mt, P)],
                            rhs=hTs[half, ko][:, ts(qc, QTOK)],
                            start=(ko == 0),
                            stop=(ko == N_KO - 1),
                        )
                    lh = lhd_p.tile([P, QTOK], bf16, name="lhd")
                    leaky(lh, psd, "dep", mt)
                    tiles[mt] = lh
                return tiles

            def dep_quarter_scores(lhq, q, dep_all):
                """Scores for quarter q -> dep_all[:, 4q:4q+4] per-token
                scalars (2 i-tiles x 2 channels)."""
                pdt = tr_ps.tile([2, QTOK], f32, name="ptr",
                                 padded_shape=[P, 512])
                for mt in range(N_MT):
                    nc.tensor.matmul(
                        pdt,
                        lhsT=wc_dep_sb[:, mt, :],
                        rhs=lhq[mt],
                        start=(mt == 0),
                        stop=(mt == N_MT - 1),
                    )
                dep_t = dept_p.tile([2, QTOK], f32r, name="dep_t")
                nc.scalar.activation(dep_t, pdt, Identity)
                pds = tr_ps.tile([P, 4], f32r, name="ptr", padded_shape=[P, 512])
                for t in range(2):
                    nc.tensor.matmul(
                        pds[:, 2 * t : 2 * t + 2],
                        lhsT=dep_t[:, ts(t, P)],
                        rhs=ident_sb[:2, :2],
                        is_transpose=True,
                        start=True,
                        stop=True,
                    )
                nc.vector.tensor_copy(dep_all[:, 4 * q : 4 * q + 4], pds)

            def asm_store(b, q, dep_all, hb_tiles, jhalves, eng):
                """Assemble + store out tiles for the 2 i-tiles of quarter q,
                covering the given j-halves (None = full row)."""
                full = jhalves is None
                for t in range(2):
                    tsub = 2 * q + t
                    d0 = dep_all[:, 4 * q + 2 * t : 4 * q + 2 * t + 1]
                    d1 = dep_all[:, 4 * q + 2 * t + 1 : 4 * q + 2 * t + 2]
                    if full:
                        ot = out_p.tile([P, L, 2], f32, name="ot")
                        for c, dap in ((0, d0), (1, d1)):
                            e = eng()
                            if e is nc.scalar:
                                e.activation(ot[:, :, c], hb_tiles[c], Identity,
                                             bias=dap)
                            else:
                                e.tensor_scalar(ot[:, :, c], hb_tiles[c], dap,
                                                None, Add)
                        nc.sync.dma_start(out_d[b, ts(tsub, P)], ot)
                    else:
                        for jh in jhalves:
                            ot = outh_p.tile([P, 512, 2], f32, name="oth")
                            for c, dap in ((0, d0), (1, d1)):
                                e = eng()
                                src = hb_tiles[c][:, ts(jh, 512)]
                                if e is nc.scalar:
                                    e.activation(ot[:, :, c], src, Identity,
                                                 bias=dap)
                                else:
                                    e.tensor_scalar(ot[:, :, c], src, dap,
                                                    None, Add)
                            nc.sync.dma_start(
                                out_d[b, ts(tsub, P), ts(jh, 512)], ot
                            )

            # ================= batch 0 =================
            # PE order: T0h0, H0h0, T0h1, H0h1, bc -- the head branch (and
            # with it the first stores) starts as soon as half-0 tokens and
            # w_head are resident; transposes of later data fill DMA waits.
            hTs0 = {}
            transposes(loaded, (0,), hTs0)
            lh_head = head_mlp(hTs0, (0,))
            transposes(loaded, (1,), hTs0)
            b1_nats = load_batch(1, fine=False)
            head_mlp(hTs0, (1,), tiles=lh_head)
            hb0 = {c: hbc_p.tile([P, L], f32, name="hb") for c in range(2)}
            head_bc_phase(lh_head, (0, 1), hb0)

            dep_all0 = depsc_p.tile([P, 4 * NQ], f32, name="dep_all")
            eng0 = eng_ring([nc.gpsimd, nc.scalar, nc.gpsimd,
                             nc.vector, nc.gpsimd, nc.scalar])
            hTs1 = {}
            lhq0 = dep_quarter_mm(hTs0, 0)
            lhq1 = dep_quarter_mm(hTs0, 1)
            dep_quarter_scores(lhq0, 0, dep_all0)
            asm_store(0, 0, dep_all0, hb0, None, eng0)
            transposes(b1_nats, (0,), hTs1)
            lhq2 = dep_quarter_mm(hTs0, 2)
            dep_quarter_scores(lhq1, 1, dep_all0)
            asm_store(0, 1, dep_all0, hb0, None, eng0)
            transposes(b1_nats, (1,), hTs1)
            lhq3 = dep_quarter_mm(hTs0, 3)
            dep_quarter_scores(lhq2, 2, dep_all0)
            asm_store(0, 2, dep_all0, hb0, None, eng0)
            dep_quarter_scores(lhq3, 3, dep_all0)
            asm_store(0, 3, dep_all0, hb0, None, eng0)

            # ================= batch 1 (last) =================
            lh_head = head_mlp(hTs1, (0,))
            hb1 = {c: hbc_p.tile([P, L], f32, name="hb") for c in range(2)}
            head_bc_phase(lh_head, (0,), hb1)

            dep_all1 = depsc_p.tile([P, 4 * NQ], f32, name="dep_all")
            eng1 = eng_ring([nc.gpsimd, nc.vector, nc.gpsimd,
                             nc.vector, nc.gpsimd, nc.scalar])
            # quarters 0-1: store the j<512 half as soon as scores land
            lhq0 = dep_quarter_mm(hTs1, 0)
            lhq1 = dep_quarter_mm(hTs1, 1)
            dep_quarter_scores(lhq0, 0, dep_all1)
            asm_store(1, 0, dep_all1, hb1, (0,), eng1)
            dep_quarter_scores(lhq1, 1, dep_all1)
            asm_store(1, 1, dep_all1, hb1, (0,), eng1)

            # second head half; then backfill j>=512 for quarters 0-1
            head_mlp(hTs1, (1,), tiles=lh_head)
            head_bc_phase(lh_head, (1,), hb1)
            asm_store(1, 0, dep_all1, hb1, (1,), eng1)
            asm_store(1, 1, dep_all1, hb1, (1,), eng1)

            # quarters 2-3: full rows
            lhq2 = dep_quarter_mm(hTs1, 2)
            dep_quarter_scores(lhq2, 2, dep_all1)
            asm_store(1, 2, dep_all1, hb1, None, eng1)
            lhq3 = dep_quarter_mm(hTs1, 3)
            dep_quarter_scores(lhq3, 3, dep_all1)
            asm_store(1, 3, dep_all1, hb1, None, eng1)

    nc.compile()
    return nc


def _prep_consts(W_dep, b_dep, W_head, b_head, Wc, bc):
    import ml_dtypes

    f = np.float32
    bf = ml_dtypes.bfloat16

    def pad_w(W):
        Wp = np.zeros((D, MLP_PAD), f)
        Wp[:, :MLP] = W
        return Wp.astype(bf)

    def bias_t(bvec):
        bp = np.zeros((MLP_PAD,), f)
        bp[:MLP] = bvec
        bt = bp.reshape(N_MT, P).T  # [P, N_MT]
        out = np.empty((P, 2 * N_MT), f)
        out[:, 0::2] = (1.0 - NEG_SLOPE) * bt
        out[:, 1::2] = bt
        return out

    wc_dep_pad = np.zeros((MLP_PAD, 2), f)
    wc_dep_pad[:MLP] = Wc[:MLP]
    wc_dep_t = wc_dep_pad.reshape(N_MT, P, 2).transpose(1, 0, 2).copy()  # [P,mt,2]

    wc_head_pad = np.zeros((MLP_PAD, 2), f)
    wc_head_pad[:MLP] = Wc[MLP:]
    wh = wc_head_pad.reshape(N_MT, P, 2).transpose(1, 2, 0)  # [P, 2, N_MT]
    wc_head_bc = np.broadcast_to(wh[:, :, :, None], (P, 2, N_MT, P)).copy()

    return {
        "w_dep": pad_w(W_dep),
        "w_head": pad_w(W_head),
        "b_dep_t": bias_t(b_dep),
        "b_head_t": bias_t(b_head),
        "wc_dep_t": wc_dep_t.astype(bf),
        "wc_head_bc": wc_head_bc.astype(bf),
        "bc_bc": np.broadcast_to(bc.astype(f), (P, 2)).copy(),
        "ident": np.eye(P, dtype=f),
    }


def kernel(hidden_state, W_dep, b_dep, W_head, b_head, Wc, bc):
    from concourse.bass_utils import run_bass_kernel_spmd

    hidden_state = np.ascontiguousarray(np.asarray(hidden_state, dtype=np.float32))
    consts = _prep_consts(
        np.asarray(W_dep, np.float32),
        np.asarray(b_dep, np.float32),
        np.asarray(W_head, np.float32),
        np.asarray(b_head, np.float32),
        np.asarray(Wc, np.float32),
        np.asarray(bc, np.float32),
    )

    if "nc" not in _CACHE:
        _CACHE["nc"] = _build_nc()
    nc = _CACHE["nc"]

    in_maps = []
    for k in range(N_CORES):
        m = {"hidden": hidden_state[k * B_PER_CORE : (k + 1) * B_PER_CORE]}
        m.update(consts)
        in_maps.append(m)

    trace = bool(int(os.environ.get("BB_TRACE", "0")))
    if not trace:
        # The NTFF profiling hook (antenv.axon_hooks) is absent in this
        # container; a stray BASS_TRACE=1 would crash the run. Force off.
        os.environ["BASS_NEVER_TRACE"] = "1"
    res = run_bass_kernel_spmd(nc, in_maps, list(range(N_CORES)), trace=trace)
    _CACHE["last_results"] = res
    out = np.concatenate([res.results[k]["out"] for k in range(N_CORES)], axis=0)
    return out


# revision 14
# speedup vs baseline: 1.0148x; 1.0148x over previous
"""Trainium2 Bass kernel for nn_BinaryBiaffine2 (biaffine dependency scorer).

Math (per batch b):
    h_dep  = leaky_relu(hidden @ W_dep  + b_dep)             [L, 500]
    h_head = leaky_relu(hidden @ W_head + b_head)            [L, 500]
    dep    = h_dep  @ Wc[:500]                               [L, 2]
    head   = h_head @ Wc[500:]                               [L, 2]
    out[i, j, c] = dep[i, c] + head[j, c] + bc[c]            [L, L, 2]

Sharding: data-parallel over batch, 2 batches per core on 8 cores.

Per-core strategy (v3):
  - hidden loaded natural ([tok, d]), PE-transposed 128x128 into hT tiles
    [d, tok] stored bf16 (identity is bf16 so transposes run at 1 cyc/row).
  - weights bf16 (halves weight HBM traffic; matmuls full-rate at any N).
  - MLP in [m, tok] layout; leaky(x+b) = relu(0.99x+0.99b) + 0.01*(x+b)
    with the linear term in bf16 so the DVE add runs in 2x mode.
  - head scores broadcast across partitions via matmul with a
    partition-replicated Wc column as stationary (+bc folded in the copy).
  - dep branch runs in token QUARTERS (256 tok) so assembly+stores stream
    at fine granularity; all DMAs (loads+stores) issue from the SP queue
    in readiness order (single shared DMA device in hw).
  - batch 1 (last) computes its head branch in halves: out[i, j in h0]
    tiles store while head-h1 is still being computed, shrinking the
    end-of-kernel store tail.
"""

import os
import sys

if "/opt/trn_rl_repo" not in sys.path:
    sys.path.insert(0, "/opt/trn_rl_repo")

import numpy as np

B, L, D = 16, 1024, 1024
MLP = 500
MLP_PAD = 512
NEG_SLOPE = 0.01
N_CORES = 8
B_PER_CORE = B // N_CORES
P = 128
N_MT = MLP_PAD // P  # 4 m-tiles of 128
N_KO = D // P        # 8 d-slices of 128
N_TSUB = L // P      # 8 token subtiles per batch
NQ = 4               # dep-branch quarters per batch (256 tok each)
QTOK = L // NQ

_CACHE = {}


def _build_nc():
    import concourse.tile as tile
    from concourse import bacc, mybir
    from concourse.bass import ts
    from contextlib import ExitStack

    f32 = mybir.dt.float32
    f32r = mybir.dt.float32r
    bf16 = mybir.dt.bfloat16
    Relu = mybir.ActivationFunctionType.Relu
    Identity = mybir.ActivationFunctionType.Identity
    Add = mybir.AluOpType.add
    Mult = mybir.AluOpType.mult

    nc = bacc.Bacc()

    hidden = nc.dram_tensor("hidden", [B_PER_CORE, L, D], f32r, kind="ExternalInput")
    w_dep_d = nc.dram_tensor("w_dep", [D, MLP_PAD], bf16, kind="ExternalInput")
    w_head_d = nc.dram_tensor("w_head", [D, MLP_PAD], bf16, kind="ExternalInput")
    # bias tiles: columns (2*mt, 2*mt+1) = (0.99*b, b) for m-tile mt
    b_dep_d = nc.dram_tensor("b_dep_t", [P, 2 * N_MT], f32, kind="ExternalInput")
    b_head_d = nc.dram_tensor("b_head_t", [P, 2 * N_MT], f32, kind="ExternalInput")
    wc_dep_d = nc.dram_tensor("wc_dep_t", [P, N_MT, 2], bf16, kind="ExternalInput")
    wc_head_d = nc.dram_tensor("wc_head_bc", [P, 2, N_MT, P], bf16, kind="ExternalInput")
    bc_d = nc.dram_tensor("bc_bc", [P, 2], f32, kind="ExternalInput")
    ident_d = nc.dram_tensor("ident", [P, P], f32r, kind="ExternalInput")
    out_d = nc.dram_tensor("out", [B_PER_CORE, L, L, 2], f32, kind="ExternalOutput")

    with tile.TileContext(nc) as tc:
        with ExitStack() as ctx:
            const = ctx.enter_context(tc.tile_pool(name="const", bufs=1))
            hnat_p = ctx.enter_context(tc.tile_pool(name="hnat", bufs=4))
            hT_p = ctx.enter_context(tc.tile_pool(name="hT", bufs=24))
            lh_p = ctx.enter_context(tc.tile_pool(name="lh", bufs=5))
            lhd_p = ctx.enter_context(tc.tile_pool(name="lhd", bufs=8))
            tmp_p = ctx.enter_context(tc.tile_pool(name="tmp", bufs=3))
            dept_p = ctx.enter_context(tc.tile_pool(name="dept", bufs=2))
            depsc_p = ctx.enter_context(tc.tile_pool(name="depsc", bufs=2))
            hbc_p = ctx.enter_context(tc.tile_pool(name="hbc", bufs=4))
            out_p = ctx.enter_context(tc.tile_pool(name="outp", bufs=5))
            outh_p = ctx.enter_context(tc.tile_pool(name="outh", bufs=6))
            tr_ps = ctx.enter_context(tc.tile_pool(name="trps", bufs=4, space="PSUM"))
            h_ps = ctx.enter_context(tc.tile_pool(name="hps", bufs=4, space="PSUM"))

            # ---- constants: tiny ones first, then batch-0 hidden, then
            # weights; all on the SP (sync) HWDGE queue which carries every
            # DMA so the shared DMA device runs them in readiness order.
            ident_sb = const.tile([P, P], f32r)
            nc.sync.dma_start(ident_sb, ident_d[:, :])

            hid_r = hidden[:, :, :]

            def load_batch(b, fine):
                h_nats = []
                for tp in range(N_TSUB // 2):
                    h_nat = hnat_p.tile([P, 2, D], f32r, name="h_nat")
                    if fine:
                        for s in range(2):
                            nc.sync.dma_start(
                                h_nat[:, s],
                                hid_r[b, ts(2 * tp + s, P), :],
                            )
                    else:
                        nc.sync.dma_start(
                            h_nat,
                            hid_r[b, ts(tp, 2 * P), :].rearrange(
                                "(s p) d -> p s d", p=P
                            ),
                        )
                    h_nats.append(h_nat)
                return h_nats

            # batch-0 half 0 tokens first, then w_head (so the head MLP can
            # start on half 0 while half 1 + w_dep are still loading)
            w_sb = {}
            loaded = []

            def load_b0_half(half):
                for tp in range(2 * half, 2 * half + 2):
                    h_nat = hnat_p.tile([P, 2, D], f32r, name="h_nat")
                    for s in range(2):
                        nc.sync.dma_start(
                            h_nat[:, s], hid_r[0, ts(2 * tp + s, P), :]
                        )
                    loaded.append(h_nat)

            load_b0_half(0)
            b_sb = {}
            b_head_sb = const.tile([P, 2 * N_MT], f32)
            nc.sync.dma_start(b_head_sb, b_head_d[:, :])
            w_head_sb = const.tile([P, N_KO, MLP_PAD], bf16)
            nc.sync.dma_start(
                w_head_sb, w_head_d[:, :].rearrange("(ko p) m -> p ko m", p=P)
            )
            load_b0_half(1)
            w_dep_sb = const.tile([P, N_KO, MLP_PAD], bf16)
            nc.sync.dma_start(
                w_dep_sb, w_dep_d[:, :].rearrange("(ko p) m -> p ko m", p=P)
            )
            w_sb["dep"], w_sb["head"] = w_dep_sb, w_head_sb
            b_dep_sb = const.tile([P, 2 * N_MT], f32)
            nc.sync.dma_start(b_dep_sb, b_dep_d[:, :])
            b_sb["dep"], b_sb["head"] = b_dep_sb, b_head_sb
            wc_dep_sb = const.tile([P, N_MT, 2], bf16)
            nc.sync.dma_start(wc_dep_sb, wc_dep_d[:, :, :])
            bc_sb = const.tile([P, 2], f32)
            nc.sync.dma_start(bc_sb, bc_d[:, :])
            wc_head_sb = const.tile([P, 2, N_MT, P], bf16)
            nc.sync.dma_start(wc_head_sb, wc_head_d[:, :, :, :])

            # rotating engine assignment for elementwise output/copy units
            def eng_ring(seq):
                i = [0]

                def nxt():
                    e = seq[i[0] % len(seq)]
                    i[0] += 1
                    return e

                return nxt

            def transposes(h_nats, halves=(0, 1), hTs=None):
                """PE-transpose a batch into hT tiles [d=128, tok=512] bf16."""
                if hTs is None:
                    hTs = {}
                cp = eng_ring([nc.vector, nc.vector, nc.scalar])
                for half in halves:
                    for ko in range(N_KO):
                        ptr = tr_ps.tile([P, 512], f32r, name="ptr")
                        for q in range(4):
                            tsub = half * 4 + q
                            nc.tensor.matmul(
                                ptr[:, ts(q, P)],
                                lhsT=h_nats[tsub // 2][:, tsub % 2, ts(ko, P)],
                                rhs=ident_sb,
                                is_transpose=True,
                                start=True,
                                stop=True,
                            )
                        hT = hT_p.tile([P, 512], bf16, name="hT")
                        e = cp()
                        if e is nc.scalar:
                            e.activation(hT, ptr, Identity)
                        else:
                            e.tensor_copy(hT, ptr)
                        hTs[half, ko] = hT
                return hTs

            def leaky(dst, ps, br, mt):
                """dst(bf16) = leaky_relu(ps + b) via relu(0.99x+0.99b) +
                0.01(x+b); the bf16 add runs in DVE 2x mode."""
                lt = tmp_p.tile(list(dst.shape), bf16, name="lt")
                nc.scalar.activation(
                    dst, ps, Relu,
                    bias=b_sb[br][:, 2 * mt : 2 * mt + 1],
                    scale=1.0 - NEG_SLOPE,
                )
                nc.vector.tensor_scalar(
                    lt, ps,
                    b_sb[br][:, 2 * mt + 1 : 2 * mt + 2], NEG_SLOPE,
                    Add, Mult,
                )
                nc.vector.tensor_add(dst, dst, lt)

            def head_mlp(hTs, halves, tiles=None):
                """Head branch for the given token halves -> lh tiles
                [m=128, L] bf16 keyed by mt (columns half*512.. filled).
                Emits all of half A before half B so the PE can run as soon
                as half A's hT tiles exist."""
                if tiles is None:
                    tiles = {mt: lh_p.tile([P, L], bf16, name="lh")
                             for mt in range(N_MT)}
                for half in halves:
                    for mt in range(N_MT):
                        ps = h_ps.tile([P, 512], f32, name="hps")
                        for ko in range(N_KO):
                            nc.tensor.matmul(
                                ps,
                                lhsT=w_sb["head"][:, ko, ts(mt, P)],
                                rhs=hTs[half, ko],
                                start=(ko == 0),
                                stop=(ko == N_KO - 1),
                            )
                        leaky(tiles[mt][:, ts(half, 512)], ps, "head", mt)
                return tiles

            def head_bc_phase(lh_tiles, halves, hb_tiles):
                """Head scores, partition-broadcast, +bc folded; writes the
                [:, half*512:...] columns of hb_tiles[c]."""
                for half in halves:
                    for c in range(2):
                        pbc = h_ps.tile([P, 512], f32, name="hps")
                        for mt in range(N_MT):
                            nc.tensor.matmul(
                                pbc,
                                lhsT=wc_head_sb[:, c, mt, :],
                                rhs=lh_tiles[mt][:, ts(half, 512)],
                                start=(mt == 0),
                                stop=(mt == N_MT - 1),
                            )
                        nc.scalar.activation(
                            hb_tiles[c][:, ts(half, 512)],
                            pbc,
                            Identity,
                            bias=bc_sb[:, c : c + 1],
                        )

            def dep_quarter_mm(hTs, q):
                """Dep-branch MLP for token quarter q -> lh tiles [m, 256]."""
                half, qc = q // 2, q % 2
                tiles = {}
                for mt in range(N_MT):
                    psd = h_ps.tile([P, QTOK], f32, name="hps",
                                    padded_shape=[P, 512])
                    for ko in range(N_KO):
                        nc.tensor.matmul(
                            psd,
                            lhsT=w_sb["dep"][:, ko, ts(mt, P)],
                            rhs=hTs[half, ko][:, ts(qc, QTOK)],
                            start=(ko == 0),
                            stop=(ko == N_KO - 1),
                        )
                    lh = lhd_p.tile([P, QTOK], bf16, name="lhd")
                    leaky(lh, psd, "dep", mt)
                    tiles[mt] = lh
                return tiles

            def dep_quarter_scores(lhq, q, dep_all):
                """Scores for quarter q -> dep_all[:, 4q:4q+4] per-token
                scalars (2 i-tiles x 2 channels)."""
                pdt = tr_ps.tile([2, QTOK], f32, name="ptr",
                                 padded_shape=[P, 512])
                for mt in range(N_MT):
                    nc.tensor.matmul(
                        pdt,
                        lhsT=wc_dep_sb[:, mt, :],
                        rhs=lhq[mt],
                        start=(mt == 0),
                        stop=(mt == N_MT - 1),
                    )
                dep_t = dept_p.tile([2, QTOK], f32r, name="dep_t")
                nc.scalar.activation(dep_t, pdt, Identity)
                pds = tr_ps.tile([P, 4], f32r, name="ptr", padded_shape=[P, 512])
                for t in range(2):
                    nc.tensor.matmul(
                        pds[:, 2 * t : 2 * t + 2],
                        lhsT=dep_t[:, ts(t, P)],
                        rhs=ident_sb[:2, :2],
                        is_transpose=True,
                        start=True,
                        stop=True,
                    )
                nc.vector.tensor_copy(dep_all[:, 4 * q : 4 * q + 4], pds)

            def asm_store(b, q, dep_all, hb_tiles, jhalves, eng):
                """Assemble + store out tiles for the 2 i-tiles of quarter q,
                covering the given j-halves (None = full row)."""
                full = jhalves is None
                for t in range(2):
                    tsub = 2 * q + t
                    d0 = dep_all[:, 4 * q + 2 * t : 4 * q + 2 * t + 1]
                    d1 = dep_all[:, 4 * q + 2 * t + 1 : 4 * q + 2 * t + 2]
                    if full:
                        ot = out_p.tile([P, L, 2], f32, name="ot")
                        for c, dap in ((0, d0), (1, d1)):
                            e = eng()
                            if e is nc.scalar:
                                e.activation(ot[:, :, c], hb_tiles[c], Identity,
                                             bias=dap)
                            else:
                                e.tensor_scalar(ot[:, :, c], hb_tiles[c], dap,
                                                None, Add)
                        nc.sync.dma_start(out_d[b, ts(tsub, P)], ot)
                    else:
                        for jh in jhalves:
                            ot = outh_p.tile([P, 512, 2], f32, name="oth")
                            for c, dap in ((0, d0), (1, d1)):
                                e = eng()
                                src = hb_tiles[c][:, ts(jh, 512)]
                                if e is nc.scalar:
                                    e.activation(ot[:, :, c], src, Identity,
                                                 bias=dap)
                                else:
                                    e.tensor_scalar(ot[:, :, c], src, dap,
                                                    None, Add)
                            nc.sync.dma_start(
                                out_d[b, ts(tsub, P), ts(jh, 512)], ot
                            )

            # ================= batch 0 =================
            # PE order: T0h0, H0h0, T0h1, H0h1, bc -- the head branch (and
            # with it the first stores) starts as soon as half-0 tokens and
            # w_head are resident; transposes of later data fill DMA waits.
            hTs0 = {}
            transposes(loaded, (0,), hTs0)
            lh_head = head_mlp(hTs0, (0,))
            transposes(loaded, (1,), hTs0)
            b1_nats = load_batch(1, fine=False)
            head_mlp(hTs0, (1,), tiles=lh_head)
            hb0 = {c: hbc_p.tile([P, L], f32, name="hb") for c in range(2)}
            head_bc_phase(lh_head, (0, 1), hb0)

            dep_all0 = depsc_p.tile([P, 4 * NQ], f32, name="dep_all")
            eng0 = eng_ring([nc.gpsimd, nc.scalar, nc.gpsimd,
                             nc.vector, nc.gpsimd, nc.scalar])
            hTs1 = {}
            lhq0 = dep_quarter_mm(hTs0, 0)
            lhq1 = dep_quarter_mm(hTs0, 1)
            dep_quarter_scores(lhq0, 0, dep_all0)
            asm_store(0, 0, dep_all0, hb0, None, eng0)
            transposes(b1_nats, (0,), hTs1)
            lhq2 = dep_quarter_mm(hTs0, 2)
            dep_quarter_scores(lhq1, 1, dep_all0)
            asm_store(0, 1, dep_all0, hb0, None, eng0)
            transposes(b1_nats, (1,), hTs1)
            lhq3 = dep_quarter_mm(hTs0, 3)
            dep_quarter_scores(lhq2, 2, dep_all0)
            asm_store(0, 2, dep_all0, hb0, None, eng0)
            dep_quarter_scores(lhq3, 3, dep_all0)
            asm_store(0, 3, dep_all0, hb0, None, eng0)

            # ================= batch 1 (last) =================
            lh_head = head_mlp(hTs1, (0,))
            hb1 = {c: hbc_p.tile([P, L], f32, name="hb") for c in range(2)}
            head_bc_phase(lh_head, (0,), hb1)

            dep_all1 = depsc_p.tile([P, 4 * NQ], f32, name="dep_all")
            eng1 = eng_ring([nc.gpsimd, nc.vector, nc.gpsimd,
                             nc.vector, nc.gpsimd, nc.scalar])
            # quarters 0-1: store the j<512 half as soon as scores land
            lhq0 = dep_quarter_mm(hTs1, 0)
            lhq1 = dep_quarter_mm(hTs1, 1)
            dep_quarter_scores(lhq0, 0, dep_all1)
            asm_store(1, 0, dep_all1, hb1, (0,), eng1)
            dep_quarter_scores(lhq1, 1, dep_all1)
            asm_store(1, 1, dep_all1, hb1, (0,), eng1)

            # second head half; then backfill j>=512 for quarters 0-1
            head_mlp(hTs1, (1,), tiles=lh_head)
            head_bc_phase(lh_head, (1,), hb1)
            asm_store(1, 0, dep_all1, hb1, (1,), eng1)
            asm_store(1, 1, dep_all1, hb1, (1,), eng1)

            # quarters 2-3: full rows
            lhq2 = dep_quarter_mm(hTs1, 2)
            dep_quarter_scores(lhq2, 2, dep_all1)
            asm_store(1, 2, dep_all1, hb1, None, eng1)
            lhq3 = dep_quarter_mm(hTs1, 3)
            dep_quarter_scores(lhq3, 3, dep_all1)
            asm_store(1, 3, dep_all1, hb1, None, eng1)

    nc.compile()
    return nc


def _prep_consts(W_dep, b_dep, W_head, b_head, Wc, bc):
    import ml_dtypes

    f = np.float32
    bf = ml_dtypes.bfloat16

    def pad_w(W):
        Wp = np.zeros((D, MLP_PAD), f)
        Wp[:, :MLP] = W
        return Wp.astype(bf)

    def bias_t(bvec):
        bp = np.zeros((MLP_PAD,), f)
        bp[:MLP] = bvec
        bt = bp.reshape(N_MT, P).T  # [P, N_MT]
        out = np.empty((P, 2 * N_MT), f)
        out[:, 0::2] = (1.0 - NEG_SLOPE) * bt
        out[:, 1::2] = bt
        return out

    wc_dep_pad = np.zeros((MLP_PAD, 2), f)
    wc_dep_pad[:MLP] = Wc[:MLP]
    wc_dep_t = wc_dep_pad.reshape(N_MT, P, 2).transpose(1, 0, 2).copy()  # [P,mt,2]

    wc_head_pad = np.zeros((MLP_PAD, 2), f)
    wc_head_pad[:MLP] = Wc[MLP:]
    wh = wc_head_pad.reshape(N_MT, P, 2).transpose(1, 2, 0)  # [P, 2, N_MT]
    wc_head_bc = np.broadcast_to(wh[:, :, :, None], (P, 2, N_MT, P)).copy()

    return {
        "w_dep": pad_w(W_dep),
        "w_head": pad_w(W_head),
        "b_dep_t": bias_t(b_dep),
        "b_head_t": bias_t(b_head),
        "wc_dep_t": wc_dep_t.astype(bf),
        "wc_head_bc": wc_head_bc.astype(bf),
        "bc_bc": np.broadcast_to(bc.astype(f), (P, 2)).copy(),
        "ident": np.eye(P, dtype=f),
    }


def kernel(hidden_state, W_dep, b_dep, W_head, b_head, Wc, bc):
    from concourse.bass_utils import run_bass_kernel_spmd

    hidden_state = np.ascontiguousarray(np.asarray(hidden_state, dtype=np.float32))
    consts = _prep_consts(
        np.asarray(W_dep, np.float32),
        np.asarray(b_dep, np.float32),
        np.asarray(W_head, np.float32),
        np.asarray(b_head, np.float32),
        np.asarray(Wc, np.float32),
        np.asarray(bc, np.float32),
    )

    if "nc" not in _CACHE:
        _CACHE["nc"] = _build_nc()
    nc = _CACHE["nc"]

    in_maps = []
    for k in range(N_CORES):
        m = {"hidden": hidden_state[k * B_PER_CORE : (k + 1) * B_PER_CORE]}
        m.update(consts)
        in_maps.append(m)

    trace = bool(int(os.environ.get("BB_TRACE", "0")))
    if not trace:
        # The NTFF profiling hook (antenv.axon_hooks) is absent in this
        # container; a stray BASS_TRACE=1 would crash the run. Force off.
        os.environ["BASS_NEVER_TRACE"] = "1"
    res = run_bass_kernel_spmd(nc, in_maps, list(range(N_CORES)), trace=trace)
    _CACHE["last_results"] = res
    out = np.concatenate([res.results[k]["out"] for k in range(N_CORES)], axis=0)
    return out


# revision 15
# speedup vs baseline: 1.0219x; 1.0070x over previous
"""Trainium2 Bass kernel for nn_BinaryBiaffine2 (biaffine dependency scorer).

Math (per batch b):
    h_dep  = leaky_relu(hidden @ W_dep  + b_dep)             [L, 500]
    h_head = leaky_relu(hidden @ W_head + b_head)            [L, 500]
    dep    = h_dep  @ Wc[:500]                               [L, 2]
    head   = h_head @ Wc[500:]                               [L, 2]
    out[i, j, c] = dep[i, c] + head[j, c] + bc[c]            [L, L, 2]

Sharding: data-parallel over batch, 2 batches per core on 8 cores.

Per-core strategy (v3):
  - hidden loaded natural ([tok, d]), PE-transposed 128x128 into hT tiles
    [d, tok] stored bf16 (identity is bf16 so transposes run at 1 cyc/row).
  - weights bf16 (halves weight HBM traffic; matmuls full-rate at any N).
  - MLP in [m, tok] layout; leaky(x+b) = relu(0.99x+0.99b) + 0.01*(x+b)
    with the linear term in bf16 so the DVE add runs in 2x mode.
  - head scores broadcast across partitions via matmul with a
    partition-replicated Wc column as stationary (+bc folded in the copy).
  - dep branch runs in token QUARTERS (256 tok) so assembly+stores stream
    at fine granularity; all DMAs (loads+stores) issue from the SP queue
    in readiness order (single shared DMA device in hw).
  - batch 1 (last) computes its head branch in halves: out[i, j in h0]
    tiles store while head-h1 is still being computed, shrinking the
    end-of-kernel store tail.
"""

import os
import sys

if "/opt/trn_rl_repo" not in sys.path:
    sys.path.insert(0, "/opt/trn_rl_repo")

import numpy as np

B, L, D = 16, 1024, 1024
MLP = 500
MLP_PAD = 512
NEG_SLOPE = 0.01
N_CORES = 8
B_PER_CORE = B // N_CORES
P = 128
N_MT = MLP_PAD // P  # 4 m-tiles of 128
N_KO = D // P        # 8 d-slices of 128
N_TSUB = L // P      # 8 token subtiles per batch
NQ = 4               # dep-branch quarters per batch (256 tok each)
QTOK = L // NQ

_CACHE = {}


def _build_nc():
    import concourse.tile as tile
    from concourse import bacc, mybir
    from concourse.bass import ts
    from contextlib import ExitStack

    f32 = mybir.dt.float32
    f32r = mybir.dt.float32r
    bf16 = mybir.dt.bfloat16
    Relu = mybir.ActivationFunctionType.Relu
    Identity = mybir.ActivationFunctionType.Identity
    Add = mybir.AluOpType.add
    Mult = mybir.AluOpType.mult

    nc = bacc.Bacc()

    hidden = nc.dram_tensor("hidden", [B_PER_CORE, L, D], f32r, kind="ExternalInput")
    w_dep_d = nc.dram_tensor("w_dep", [D, MLP_PAD], bf16, kind="ExternalInput")
    w_head_d = nc.dram_tensor("w_head", [D, MLP_PAD], bf16, kind="ExternalInput")
    # bias tiles: columns (2*mt, 2*mt+1) = (0.99*b, b) for m-tile mt
    b_dep_d = nc.dram_tensor("b_dep_t", [P, 2 * N_MT], f32, kind="ExternalInput")
    b_head_d = nc.dram_tensor("b_head_t", [P, 2 * N_MT], f32, kind="ExternalInput")
    wc_dep_d = nc.dram_tensor("wc_dep_t", [P, N_MT, 2], bf16, kind="ExternalInput")
    wc_head_d = nc.dram_tensor("wc_head_bc", [P, 2, N_MT, P], bf16, kind="ExternalInput")
    bc_d = nc.dram_tensor("bc_bc", [P, 2], f32, kind="ExternalInput")
    ident_d = nc.dram_tensor("ident", [P, P], f32r, kind="ExternalInput")
    out_d = nc.dram_tensor("out", [B_PER_CORE, L, L, 2], f32, kind="ExternalOutput")

    with tile.TileContext(nc) as tc:
        with ExitStack() as ctx:
            const = ctx.enter_context(tc.tile_pool(name="const", bufs=1))
            hnat_p = ctx.enter_context(tc.tile_pool(name="hnat", bufs=4))
            hT_p = ctx.enter_context(tc.tile_pool(name="hT", bufs=24))
            lh_p = ctx.enter_context(tc.tile_pool(name="lh", bufs=5))
            lhd_p = ctx.enter_context(tc.tile_pool(name="lhd", bufs=8))
            tmp_p = ctx.enter_context(tc.tile_pool(name="tmp", bufs=3))
            dept_p = ctx.enter_context(tc.tile_pool(name="dept", bufs=2))
            depsc_p = ctx.enter_context(tc.tile_pool(name="depsc", bufs=2))
            hbc_p = ctx.enter_context(tc.tile_pool(name="hbc", bufs=4))
            out_p = ctx.enter_context(tc.tile_pool(name="outp", bufs=5))
            outh_p = ctx.enter_context(tc.tile_pool(name="outh", bufs=6))
            tr_ps = ctx.enter_context(tc.tile_pool(name="trps", bufs=4, space="PSUM"))
            h_ps = ctx.enter_context(tc.tile_pool(name="hps", bufs=4, space="PSUM"))

            # ---- constants: tiny ones first, then batch-0 hidden, then
            # weights; all on the SP (sync) HWDGE queue which carries every
            # DMA so the shared DMA device runs them in readiness order.
            ident_sb = const.tile([P, P], f32r)
            nc.sync.dma_start(ident_sb, ident_d[:, :])

            hid_r = hidden[:, :, :]

            def load_batch(b, fine):
                h_nats = []
                for tp in range(N_TSUB // 2):
                    h_nat = hnat_p.tile([P, 2, D], f32r, name="h_nat")
                    if fine:
                        for s in range(2):
                            nc.sync.dma_start(
                                h_nat[:, s],
                                hid_r[b, ts(2 * tp + s, P), :],
                            )
                    else:
                        nc.sync.dma_start(
                            h_nat,
                            hid_r[b, ts(tp, 2 * P), :].rearrange(
                                "(s p) d -> p s d", p=P
                            ),
                        )
                    h_nats.append(h_nat)
                return h_nats

            # batch-0 half 0 tokens first, then w_head (so the head MLP can
            # start on half 0 while half 1 + w_dep are still loading)
            w_sb = {}
            loaded = []

            def load_b0_half(half):
                for tp in range(2 * half, 2 * half + 2):
                    h_nat = hnat_p.tile([P, 2, D], f32r, name="h_nat")
                    for s in range(2):
                        nc.sync.dma_start(
                            h_nat[:, s], hid_r[0, ts(2 * tp + s, P), :]
                        )
                    loaded.append(h_nat)

            load_b0_half(0)
            b_sb = {}
            w_head_sb = const.tile([P, N_KO, MLP_PAD], bf16)
            nc.sync.dma_start(
                w_head_sb, w_head_d[:, :].rearrange("(ko p) m -> p ko m", p=P)
            )
            b_head_sb = const.tile([P, 2 * N_MT], f32)
            nc.sync.dma_start(b_head_sb, b_head_d[:, :])
            load_b0_half(1)
            w_dep_sb = const.tile([P, N_KO, MLP_PAD], bf16)
            nc.sync.dma_start(
                w_dep_sb, w_dep_d[:, :].rearrange("(ko p) m -> p ko m", p=P)
            )
            w_sb["dep"], w_sb["head"] = w_dep_sb, w_head_sb
            b_dep_sb = const.tile([P, 2 * N_MT], f32)
            nc.sync.dma_start(b_dep_sb, b_dep_d[:, :])
            b_sb["dep"], b_sb["head"] = b_dep_sb, b_head_sb
            wc_dep_sb = const.tile([P, N_MT, 2], bf16)
            nc.sync.dma_start(wc_dep_sb, wc_dep_d[:, :, :])
            bc_sb = const.tile([P, 2], f32)
            nc.sync.dma_start(bc_sb, bc_d[:, :])
            wc_head_sb = const.tile([P, 2, N_MT, P], bf16)
            nc.sync.dma_start(wc_head_sb, wc_head_d[:, :, :, :])

            # rotating engine assignment for elementwise output/copy units
            def eng_ring(seq):
                i = [0]

                def nxt():
                    e = seq[i[0] % len(seq)]
                    i[0] += 1
                    return e

                return nxt

            def transposes(h_nats, halves=(0, 1), hTs=None):
                """PE-transpose a batch into hT tiles [d=128, tok=512] bf16."""
                if hTs is None:
                    hTs = {}
                cp = eng_ring([nc.vector, nc.vector, nc.scalar])
                for half in halves:
                    for ko in range(N_KO):
                        ptr = tr_ps.tile([P, 512], f32r, name="ptr")
                        for q in range(4):
                            tsub = half * 4 + q
                            nc.tensor.matmul(
                                ptr[:, ts(q, P)],
                                lhsT=h_nats[tsub // 2][:, tsub % 2, ts(ko, P)],
                                rhs=ident_sb,
                                is_transpose=True,
                                start=True,
                                stop=True,
                            )
                        hT = hT_p.tile([P, 512], bf16, name="hT")
                        e = cp()
                        if e is nc.scalar:
                            e.activation(hT, ptr, Identity)
                        else:
                            e.tensor_copy(hT, ptr)
                        hTs[half, ko] = hT
                return hTs

            def leaky(dst, ps, br, mt):
                """dst(bf16) = leaky_relu(ps + b) via relu(0.99x+0.99b) +
                0.01(x+b); the bf16 add runs in DVE 2x mode."""
                lt = tmp_p.tile(list(dst.shape), bf16, name="lt")
                nc.scalar.activation(
                    dst, ps, Relu,
                    bias=b_sb[br][:, 2 * mt : 2 * mt + 1],
                    scale=1.0 - NEG_SLOPE,
                )
                nc.vector.tensor_scalar(
                    lt, ps,
                    b_sb[br][:, 2 * mt + 1 : 2 * mt + 2], NEG_SLOPE,
                    Add, Mult,
                )
                nc.vector.tensor_add(dst, dst, lt)

            def head_mlp(hTs, halves, tiles=None):
                """Head branch for the given token halves -> lh tiles
                [m=128, L] bf16 keyed by mt (columns half*512.. filled).
                Emits all of half A before half B so the PE can run as soon
                as half A's hT tiles exist."""
                if tiles is None:
                    tiles = {mt: lh_p.tile([P, L], bf16, name="lh")
                             for mt in range(N_MT)}
                for half in halves:
                    for mt in range(N_MT):
                        ps = h_ps.tile([P, 512], f32, name="hps")
                        for ko in range(N_KO):
                            nc.tensor.matmul(
                                ps,
                                lhsT=w_sb["head"][:, ko, ts(mt, P)],
                                rhs=hTs[half, ko],
                                start=(ko == 0),
                                stop=(ko == N_KO - 1),
                            )
                        leaky(tiles[mt][:, ts(half, 512)], ps, "head", mt)
                return tiles

            def head_bc_phase(lh_tiles, halves, hb_tiles):
                """Head scores, partition-broadcast, +bc folded; writes the
                [:, half*512:...] columns of hb_tiles[c]."""
                for half in halves:
                    for c in range(2):
                        pbc = h_ps.tile([P, 512], f32, name="hps")
                        for mt in range(N_MT):
                            nc.tensor.matmul(
                                pbc,
                                lhsT=wc_head_sb[:, c, mt, :],
                                rhs=lh_tiles[mt][:, ts(half, 512)],
                                start=(mt == 0),
                                stop=(mt == N_MT - 1),
                            )
                        nc.scalar.activation(
                            hb_tiles[c][:, ts(half, 512)],
                            pbc,
                            Identity,
                            bias=bc_sb[:, c : c + 1],
                        )

            def dep_quarter_mm(hTs, q):
                """Dep-branch MLP for token quarter q -> lh tiles [m, 256]."""
                half, qc = q // 2, q % 2
                tiles = {}
                for mt in range(N_MT):
                    psd = h_ps.tile([P, QTOK], f32, name="hps",
                                    padded_shape=[P, 512])
                    for ko in range(N_KO):
                        nc.tensor.matmul(
                            psd,
                            lhsT=w_sb["dep"][:, ko, ts(mt, P)],
                            rhs=hTs[half, ko][:, ts(qc, QTOK)],
                            start=(ko == 0),
                            stop=(ko == N_KO - 1),
                        )
                    lh = lhd_p.tile([P, QTOK], bf16, name="lhd")
                    leaky(lh, psd, "dep", mt)
                    tiles[mt] = lh
                return tiles

            def dep_quarter_scores(lhq, q, dep_all):
                """Scores for quarter q -> dep_all[:, 4q:4q+4] per-token
                scalars (2 i-tiles x 2 channels)."""
                pdt = tr_ps.tile([2, QTOK], f32, name="ptr",
                                 padded_shape=[P, 512])
                for mt in range(N_MT):
                    nc.tensor.matmul(
                        pdt,
                        lhsT=wc_dep_sb[:, mt, :],
                        rhs=lhq[mt],
                        start=(mt == 0),
                        stop=(mt == N_MT - 1),
                    )
                dep_t = dept_p.tile([2, QTOK], f32r, name="dep_t")
                nc.scalar.activation(dep_t, pdt, Identity)
                pds = tr_ps.tile([P, 4], f32r, name="ptr", padded_shape=[P, 512])
                for t in range(2):
                    nc.tensor.matmul(
                        pds[:, 2 * t : 2 * t + 2],
                        lhsT=dep_t[:, ts(t, P)],
                        rhs=ident_sb[:2, :2],
                        is_transpose=True,
                        start=True,
                        stop=True,
                    )
                nc.vector.tensor_copy(dep_all[:, 4 * q : 4 * q + 4], pds)

            def asm_store(b, q, dep_all, hb_tiles, jhalves, eng):
                """Assemble + store out tiles for the 2 i-tiles of quarter q,
                covering the given j-halves (None = full row)."""
                full = jhalves is None
                for t in range(2):
                    tsub = 2 * q + t
                    d0 = dep_all[:, 4 * q + 2 * t : 4 * q + 2 * t + 1]
                    d1 = dep_all[:, 4 * q + 2 * t + 1 : 4 * q + 2 * t + 2]
                    if full:
                        ot = out_p.tile([P, L, 2], f32, name="ot")
                        for c, dap in ((0, d0), (1, d1)):
                            e = eng()
                            if e is nc.scalar:
                                e.activation(ot[:, :, c], hb_tiles[c], Identity,
                                             bias=dap)
                            else:
                                e.tensor_scalar(ot[:, :, c], hb_tiles[c], dap,
                                                None, Add)
                        nc.sync.dma_start(out_d[b, ts(tsub, P)], ot)
                    else:
                        for jh in jhalves:
                            ot = outh_p.tile([P, 512, 2], f32, name="oth")
                            for c, dap in ((0, d0), (1, d1)):
                                e = eng()
                                src = hb_tiles[c][:, ts(jh, 512)]
                                if e is nc.scalar:
                                    e.activation(ot[:, :, c], src, Identity,
                                                 bias=dap)
                                else:
                                    e.tensor_scalar(ot[:, :, c], src, dap,
                                                    None, Add)
                            nc.sync.dma_start(
                                out_d[b, ts(tsub, P), ts(jh, 512)], ot
                            )

            # ================= batch 0 =================
            # PE order: T0h0, H0h0, T0h1, H0h1, bc -- the head branch (and
            # with it the first stores) starts as soon as half-0 tokens and
            # w_head are resident; transposes of later data fill DMA waits.
            hTs0 = {}
            transposes(loaded, (0,), hTs0)
            lh_head = head_mlp(hTs0, (0,))
            transposes(loaded, (1,), hTs0)
            b1_nats = load_batch(1, fine=False)
            head_mlp(hTs0, (1,), tiles=lh_head)
            hb0 = {c: hbc_p.tile([P, L], f32, name="hb") for c in range(2)}
            head_bc_phase(lh_head, (0, 1), hb0)

            dep_all0 = depsc_p.tile([P, 4 * NQ], f32, name="dep_all")
            hTs1 = {}
            lhq0 = dep_quarter_mm(hTs0, 0)
            lhq1 = dep_quarter_mm(hTs0, 1)
            dep_quarter_scores(lhq0, 0, dep_all0)
            asm_store(0, 0, dep_all0, hb0, None,
                      eng_ring([nc.scalar, nc.vector]))
            transposes(b1_nats, (0,), hTs1)
            lhq2 = dep_quarter_mm(hTs0, 2)
            dep_quarter_scores(lhq1, 1, dep_all0)
            asm_store(0, 1, dep_all0, hb0, None,
                      eng_ring([nc.gpsimd, nc.scalar, nc.vector, nc.gpsimd]))
            transposes(b1_nats, (1,), hTs1)
            lhq3 = dep_quarter_mm(hTs0, 3)
            dep_quarter_scores(lhq2, 2, dep_all0)
            asm_store(0, 2, dep_all0, hb0, None,
                      eng_ring([nc.gpsimd, nc.vector, nc.scalar, nc.gpsimd]))
            dep_quarter_scores(lhq3, 3, dep_all0)
            asm_store(0, 3, dep_all0, hb0, None,
                      eng_ring([nc.gpsimd, nc.scalar, nc.gpsimd, nc.vector]))

            # ================= batch 1 (last) =================
            lh_head = head_mlp(hTs1, (0,))
            hb1 = {c: hbc_p.tile([P, L], f32, name="hb") for c in range(2)}
            head_bc_phase(lh_head, (0,), hb1)

            dep_all1 = depsc_p.tile([P, 4 * NQ], f32, name="dep_all")
            # quarters 0-1: store the j<512 half as soon as scores land
            lhq0 = dep_quarter_mm(hTs1, 0)
            lhq1 = dep_quarter_mm(hTs1, 1)
            dep_quarter_scores(lhq0, 0, dep_all1)
            asm_store(1, 0, dep_all1, hb1, (0,),
                      eng_ring([nc.scalar, nc.vector]))
            dep_quarter_scores(lhq1, 1, dep_all1)
            asm_store(1, 1, dep_all1, hb1, (0,),
                      eng_ring([nc.gpsimd, nc.vector, nc.gpsimd, nc.scalar]))

            # second head half; then backfill j>=512 for quarters 0-1
            head_mlp(hTs1, (1,), tiles=lh_head)
            head_bc_phase(lh_head, (1,), hb1)
            asm_store(1, 0, dep_all1, hb1, (1,),
                      eng_ring([nc.scalar, nc.vector, nc.gpsimd, nc.gpsimd]))
            asm_store(1, 1, dep_all1, hb1, (1,),
                      eng_ring([nc.gpsimd, nc.vector, nc.gpsimd, nc.scalar]))

            # quarters 2-3: full rows, scores pipelined one behind the mm
            lhq2 = dep_quarter_mm(hTs1, 2)
            lhq3 = dep_quarter_mm(hTs1, 3)
            dep_quarter_scores(lhq2, 2, dep_all1)
            asm_store(1, 2, dep_all1, hb1, None,
                      eng_ring([nc.scalar, nc.vector, nc.gpsimd, nc.gpsimd]))
            dep_quarter_scores(lhq3, 3, dep_all1)
            asm_store(1, 3, dep_all1, hb1, None,
                      eng_ring([nc.vector, nc.scalar, nc.gpsimd, nc.gpsimd]))

    nc.compile()
    return nc


def _prep_consts(W_dep, b_dep, W_head, b_head, Wc, bc):
    import ml_dtypes

    f = np.float32
    bf = ml_dtypes.bfloat16

    def pad_w(W):
        Wp = np.zeros((D, MLP_PAD), f)
        Wp[:, :MLP] = W
        return Wp.astype(bf)

    def bias_t(bvec):
        bp = np.zeros((MLP_PAD,), f)
        bp[:MLP] = bvec
        bt = bp.reshape(N_MT, P).T  # [P, N_MT]
        out = np.empty((P, 2 * N_MT), f)
        out[:, 0::2] = (1.0 - NEG_SLOPE) * bt
        out[:, 1::2] = bt
        return out

    wc_dep_pad = np.zeros((MLP_PAD, 2), f)
    wc_dep_pad[:MLP] = Wc[:MLP]
    wc_dep_t = wc_dep_pad.reshape(N_MT, P, 2).transpose(1, 0, 2).copy()  # [P,mt,2]

    wc_head_pad = np.zeros((MLP_PAD, 2), f)
    wc_head_pad[:MLP] = Wc[MLP:]
    wh = wc_head_pad.reshape(N_MT, P, 2).transpose(1, 2, 0)  # [P, 2, N_MT]
    wc_head_bc = np.broadcast_to(wh[:, :, :, None], (P, 2, N_MT, P)).copy()

    return {
        "w_dep": pad_w(W_dep),
        "w_head": pad_w(W_head),
        "b_dep_t": bias_t(b_dep),
        "b_head_t": bias_t(b_head),
        "wc_dep_t": wc_dep_t.astype(bf),
        "wc_head_bc": wc_head_bc.astype(bf),
        "bc_bc": np.broadcast_to(bc.astype(f), (P, 2)).copy(),
        "ident": np.eye(P, dtype=f),
    }


def kernel(hidden_state, W_dep, b_dep, W_head, b_head, Wc, bc):
    from concourse.bass_utils import run_bass_kernel_spmd

    hidden_state = np.ascontiguousarray(np.asarray(hidden_state, dtype=np.float32))
    consts = _prep_consts(
        np.asarray(W_dep, np.float32),
        np.asarray(b_dep, np.float32),
        np.asarray(W_head, np.float32),
        np.asarray(b_head, np.float32),
        np.asarray(Wc, np.float32),
        np.asarray(bc, np.float32),
    )

    if "nc" not in _CACHE:
        _CACHE["nc"] = _build_nc()
    nc = _CACHE["nc"]

    in_maps = []
    for k in range(N_CORES):
        m = {"hidden": hidden_state[k * B_PER_CORE : (k + 1) * B_PER_CORE]}
        m.update(consts)
        in_maps.append(m)

    trace = bool(int(os.environ.get("BB_TRACE", "0")))
    if not trace:
        # The NTFF profiling hook (antenv.axon_hooks) is absent in this
        # container; a stray BASS_TRACE=1 would crash the run. Force off.
        os.environ["BASS_NEVER_TRACE"] = "1"
    res = run_bass_kernel_spmd(nc, in_maps, list(range(N_CORES)), trace=trace)
    _CACHE["last_results"] = res
    out = np.concatenate([res.results[k]["out"] for k in range(N_CORES)], axis=0)
    return out


# revision 16
# speedup vs baseline: 1.0250x; 1.0030x over previous
"""Trainium2 Bass kernel for nn_BinaryBiaffine2 (biaffine dependency scorer).

Math (per batch b):
    h_dep  = leaky_relu(hidden @ W_dep  + b_dep)             [L, 500]
    h_head = leaky_relu(hidden @ W_head + b_head)            [L, 500]
    dep    = h_dep  @ Wc[:500]                               [L, 2]
    head   = h_head @ Wc[500:]                               [L, 2]
    out[i, j, c] = dep[i, c] + head[j, c] + bc[c]            [L, L, 2]

Sharding: data-parallel over batch, 2 batches per core on 8 cores.

Per-core strategy (v3):
  - hidden loaded natural ([tok, d]), PE-transposed 128x128 into hT tiles
    [d, tok] stored bf16 (identity is bf16 so transposes run at 1 cyc/row).
  - weights bf16 (halves weight HBM traffic; matmuls full-rate at any N).
  - MLP in [m, tok] layout; leaky(x+b) = relu(0.99x+0.99b) + 0.01*(x+b)
    with the linear term in bf16 so the DVE add runs in 2x mode.
  - head scores broadcast across partitions via matmul with a
    partition-replicated Wc column as stationary (+bc folded in the copy).
  - dep branch runs in token QUARTERS (256 tok) so assembly+stores stream
    at fine granularity; all DMAs (loads+stores) issue from the SP queue
    in readiness order (single shared DMA device in hw).
  - batch 1 (last) computes its head branch in halves: out[i, j in h0]
    tiles store while head-h1 is still being computed, shrinking the
    end-of-kernel store tail.
"""

import os
import sys

if "/opt/trn_rl_repo" not in sys.path:
    sys.path.insert(0, "/opt/trn_rl_repo")

import numpy as np

B, L, D = 16, 1024, 1024
MLP = 500
MLP_PAD = 512
NEG_SLOPE = 0.01
N_CORES = 8
B_PER_CORE = B // N_CORES
P = 128
N_MT = MLP_PAD // P  # 4 m-tiles of 128
N_KO = D // P        # 8 d-slices of 128
N_TSUB = L // P      # 8 token subtiles per batch
NQ = 4               # dep-branch quarters per batch (256 tok each)
QTOK = L // NQ

_CACHE = {}


def _build_nc():
    import concourse.tile as tile
    from concourse import bacc, mybir
    from concourse.bass import ts
    from contextlib import ExitStack

    f32 = mybir.dt.float32
    f32r = mybir.dt.float32r
    bf16 = mybir.dt.bfloat16
    Relu = mybir.ActivationFunctionType.Relu
    Identity = mybir.ActivationFunctionType.Identity
    Add = mybir.AluOpType.add
    Mult = mybir.AluOpType.mult

    nc = bacc.Bacc()

    hidden = nc.dram_tensor("hidden", [B_PER_CORE, L, D], f32r, kind="ExternalInput")
    w_dep_d = nc.dram_tensor("w_dep", [D, MLP_PAD], bf16, kind="ExternalInput")
    w_head_d = nc.dram_tensor("w_head", [D, MLP_PAD], bf16, kind="ExternalInput")
    # bias tiles: columns (2*mt, 2*mt+1) = (0.99*b, b) for m-tile mt
    b_dep_d = nc.dram_tensor("b_dep_t", [P, 2 * N_MT], f32, kind="ExternalInput")
    b_head_d = nc.dram_tensor("b_head_t", [P, 2 * N_MT], f32, kind="ExternalInput")
    wc_dep_d = nc.dram_tensor("wc_dep_t", [P, N_MT, 2], bf16, kind="ExternalInput")
    wc_head_d = nc.dram_tensor("wc_head_bc", [P, 2, N_MT, P], bf16, kind="ExternalInput")
    bc_d = nc.dram_tensor("bc_bc", [P, 2], f32, kind="ExternalInput")
    ident_d = nc.dram_tensor("ident", [P, P], f32r, kind="ExternalInput")
    out_d = nc.dram_tensor("out", [B_PER_CORE, L, L, 2], f32, kind="ExternalOutput")

    with tile.TileContext(nc) as tc:
        with ExitStack() as ctx:
            const = ctx.enter_context(tc.tile_pool(name="const", bufs=1))
            hnat_p = ctx.enter_context(tc.tile_pool(name="hnat", bufs=4))
            hT_p = ctx.enter_context(tc.tile_pool(name="hT", bufs=24))
            lh_p = ctx.enter_context(tc.tile_pool(name="lh", bufs=5))
            lhd_p = ctx.enter_context(tc.tile_pool(name="lhd", bufs=8))
            tmp_p = ctx.enter_context(tc.tile_pool(name="tmp", bufs=3))
            dept_p = ctx.enter_context(tc.tile_pool(name="dept", bufs=2))
            depsc_p = ctx.enter_context(tc.tile_pool(name="depsc", bufs=2))
            hbc_p = ctx.enter_context(tc.tile_pool(name="hbc", bufs=4))
            outh_p = ctx.enter_context(tc.tile_pool(name="outh", bufs=10))
            tr_ps = ctx.enter_context(tc.tile_pool(name="trps", bufs=4, space="PSUM"))
            h_ps = ctx.enter_context(tc.tile_pool(name="hps", bufs=4, space="PSUM"))

            # ---- constants: tiny ones first, then batch-0 hidden, then
            # weights; all on the SP (sync) HWDGE queue which carries every
            # DMA so the shared DMA device runs them in readiness order.
            ident_sb = const.tile([P, P], f32r)
            nc.sync.dma_start(ident_sb, ident_d[:, :])

            hid_r = hidden[:, :, :]

            def load_batch(b, fine):
                h_nats = []
                for tp in range(N_TSUB // 2):
                    h_nat = hnat_p.tile([P, 2, D], f32r, name="h_nat")
                    if fine:
                        for s in range(2):
                            nc.sync.dma_start(
                                h_nat[:, s],
                                hid_r[b, ts(2 * tp + s, P), :],
                            )
                    else:
                        nc.sync.dma_start(
                            h_nat,
                            hid_r[b, ts(tp, 2 * P), :].rearrange(
                                "(s p) d -> p s d", p=P
                            ),
                        )
                    h_nats.append(h_nat)
                return h_nats

            # batch-0 half 0 tokens first, then w_head (so the head MLP can
            # start on half 0 while half 1 + w_dep are still loading)
            w_sb = {}
            loaded = []

            def load_b0_half(half):
                for tp in range(2 * half, 2 * half + 2):
                    h_nat = hnat_p.tile([P, 2, D], f32r, name="h_nat")
                    for s in range(2):
                        nc.sync.dma_start(
                            h_nat[:, s], hid_r[0, ts(2 * tp + s, P), :]
                        )
                    loaded.append(h_nat)

            load_b0_half(0)
            b_sb = {}
            w_head_sb = const.tile([P, N_KO, MLP_PAD], bf16)
            nc.sync.dma_start(
                w_head_sb, w_head_d[:, :].rearrange("(ko p) m -> p ko m", p=P)
            )
            b_head_sb = const.tile([P, 2 * N_MT], f32)
            nc.sync.dma_start(b_head_sb, b_head_d[:, :])
            load_b0_half(1)
            w_dep_sb = const.tile([P, N_KO, MLP_PAD], bf16)
            nc.sync.dma_start(
                w_dep_sb, w_dep_d[:, :].rearrange("(ko p) m -> p ko m", p=P)
            )
            w_sb["dep"], w_sb["head"] = w_dep_sb, w_head_sb
            b_dep_sb = const.tile([P, 2 * N_MT], f32)
            nc.sync.dma_start(b_dep_sb, b_dep_d[:, :])
            b_sb["dep"], b_sb["head"] = b_dep_sb, b_head_sb
            wc_dep_sb = const.tile([P, N_MT, 2], bf16)
            nc.sync.dma_start(wc_dep_sb, wc_dep_d[:, :, :])
            bc_sb = const.tile([P, 2], f32)
            nc.sync.dma_start(bc_sb, bc_d[:, :])
            wc_head_sb = const.tile([P, 2, N_MT, P], bf16)
            nc.sync.dma_start(wc_head_sb, wc_head_d[:, :, :, :])

            # rotating engine assignment for elementwise output/copy units
            def eng_ring(seq):
                i = [0]

                def nxt():
                    e = seq[i[0] % len(seq)]
                    i[0] += 1
                    return e

                return nxt

            def transposes(h_nats, halves=(0, 1), hTs=None):
                """PE-transpose a batch into hT tiles [d=128, tok=512] bf16."""
                if hTs is None:
                    hTs = {}
                cp = eng_ring([nc.vector, nc.vector, nc.scalar])
                for half in halves:
                    for ko in range(N_KO):
                        ptr = tr_ps.tile([P, 512], f32r, name="ptr")
                        for q in range(4):
                            tsub = half * 4 + q
                            nc.tensor.matmul(
                                ptr[:, ts(q, P)],
                                lhsT=h_nats[tsub // 2][:, tsub % 2, ts(ko, P)],
                                rhs=ident_sb,
                                is_transpose=True,
                                start=True,
                                stop=True,
                            )
                        hT = hT_p.tile([P, 512], bf16, name="hT")
                        e = cp()
                        if e is nc.scalar:
                            e.activation(hT, ptr, Identity)
                        else:
                            e.tensor_copy(hT, ptr)
                        hTs[half, ko] = hT
                return hTs

            def leaky(dst, ps, br, mt):
                """dst(bf16) = leaky_relu(ps + b) via relu(0.99x+0.99b) +
                0.01(x+b); the bf16 add runs in DVE 2x mode."""
                lt = tmp_p.tile(list(dst.shape), bf16, name="lt")
                nc.scalar.activation(
                    dst, ps, Relu,
                    bias=b_sb[br][:, 2 * mt : 2 * mt + 1],
                    scale=1.0 - NEG_SLOPE,
                )
                nc.vector.tensor_scalar(
                    lt, ps,
                    b_sb[br][:, 2 * mt + 1 : 2 * mt + 2], NEG_SLOPE,
                    Add, Mult,
                )
                nc.vector.tensor_add(dst, dst, lt)

            def head_mlp(hTs, halves, tiles=None):
                """Head branch for the given token halves -> lh tiles
                [m=128, L] bf16 keyed by mt (columns half*512.. filled).
                Emits all of half A before half B so the PE can run as soon
                as half A's hT tiles exist."""
                if tiles is None:
                    tiles = {mt: lh_p.tile([P, L], bf16, name="lh")
                             for mt in range(N_MT)}
                for half in halves:
                    for mt in range(N_MT):
                        ps = h_ps.tile([P, 512], f32, name="hps")
                        for ko in range(N_KO):
                            nc.tensor.matmul(
                                ps,
                                lhsT=w_sb["head"][:, ko, ts(mt, P)],
                                rhs=hTs[half, ko],
                                start=(ko == 0),
                                stop=(ko == N_KO - 1),
                            )
                        leaky(tiles[mt][:, ts(half, 512)], ps, "head", mt)
                return tiles

            def head_bc_phase(lh_tiles, halves, hb_tiles):
                """Head scores, partition-broadcast, +bc folded; writes the
                [:, half*512:...] columns of hb_tiles[c]."""
                for half in halves:
                    for c in range(2):
                        pbc = h_ps.tile([P, 512], f32, name="hps")
                        for mt in range(N_MT):
                            nc.tensor.matmul(
                                pbc,
                                lhsT=wc_head_sb[:, c, mt, :],
                                rhs=lh_tiles[mt][:, ts(half, 512)],
                                start=(mt == 0),
                                stop=(mt == N_MT - 1),
                            )
                        nc.scalar.activation(
                            hb_tiles[c][:, ts(half, 512)],
                            pbc,
                            Identity,
                            bias=bc_sb[:, c : c + 1],
                        )

            def dep_quarter_mm(hTs, q):
                """Dep-branch MLP for token quarter q -> lh tiles [m, 256]."""
                half, qc = q // 2, q % 2
                tiles = {}
                for mt in range(N_MT):
                    psd = h_ps.tile([P, QTOK], f32, name="hps",
                                    padded_shape=[P, 512])
                    for ko in range(N_KO):
                        nc.tensor.matmul(
                            psd,
                            lhsT=w_sb["dep"][:, ko, ts(mt, P)],
                            rhs=hTs[half, ko][:, ts(qc, QTOK)],
                            start=(ko == 0),
                            stop=(ko == N_KO - 1),
                        )
                    lh = lhd_p.tile([P, QTOK], bf16, name="lhd")
                    leaky(lh, psd, "dep", mt)
                    tiles[mt] = lh
                return tiles

            def dep_quarter_scores(lhq, q, dep_all):
                """Scores for quarter q -> dep_all[:, 4q:4q+4] per-token
                scalars (2 i-tiles x 2 channels)."""
                pdt = tr_ps.tile([2, QTOK], f32, name="ptr",
                                 padded_shape=[P, 512])
                for mt in range(N_MT):
                    nc.tensor.matmul(
                        pdt,
                        lhsT=wc_dep_sb[:, mt, :],
                        rhs=lhq[mt],
                        start=(mt == 0),
                        stop=(mt == N_MT - 1),
                    )
                dep_t = dept_p.tile([2, QTOK], f32r, name="dep_t")
                nc.scalar.activation(dep_t, pdt, Identity)
                pds = tr_ps.tile([P, 4], f32r, name="ptr", padded_shape=[P, 512])
                for t in range(2):
                    nc.tensor.matmul(
                        pds[:, 2 * t : 2 * t + 2],
                        lhsT=dep_t[:, ts(t, P)],
                        rhs=ident_sb[:2, :2],
                        is_transpose=True,
                        start=True,
                        stop=True,
                    )
                nc.vector.tensor_copy(dep_all[:, 4 * q : 4 * q + 4], pds)

            def asm_store(b, q, dep_all, hb_tiles, jhalves, eng):
                """Assemble + store out tiles for the 2 i-tiles of quarter q,
                covering the given j-halves (None = both)."""
                if jhalves is None:
                    jhalves = (0, 1)
                for t in range(2):
                    tsub = 2 * q + t
                    d0 = dep_all[:, 4 * q + 2 * t : 4 * q + 2 * t + 1]
                    d1 = dep_all[:, 4 * q + 2 * t + 1 : 4 * q + 2 * t + 2]
                    for jh in jhalves:
                        ot = outh_p.tile([P, 512, 2], f32, name="oth")
                        for c, dap in ((0, d0), (1, d1)):
                            e = eng()
                            src = hb_tiles[c][:, ts(jh, 512)]
                            if e is nc.scalar:
                                e.activation(ot[:, :, c], src, Identity,
                                             bias=dap)
                            else:
                                e.tensor_scalar(ot[:, :, c], src, dap,
                                                None, Add)
                        nc.sync.dma_start(
                            out_d[b, ts(tsub, P), ts(jh, 512)], ot
                        )

            # ================= batch 0 =================
            # PE order: T0h0, H0h0, T0h1, H0h1, bc -- the head branch (and
            # with it the first stores) starts as soon as half-0 tokens and
            # w_head are resident; transposes of later data fill DMA waits.
            hTs0 = {}
            transposes(loaded, (0,), hTs0)
            lh_head = head_mlp(hTs0, (0,))
            transposes(loaded, (1,), hTs0)
            b1_nats = load_batch(1, fine=False)
            head_mlp(hTs0, (1,), tiles=lh_head)
            hb0 = {c: hbc_p.tile([P, L], f32, name="hb") for c in range(2)}
            head_bc_phase(lh_head, (0, 1), hb0)

            dep_all0 = depsc_p.tile([P, 4 * NQ], f32, name="dep_all")
            hTs1 = {}
            lhq0 = dep_quarter_mm(hTs0, 0)
            lhq1 = dep_quarter_mm(hTs0, 1)
            dep_quarter_scores(lhq0, 0, dep_all0)
            asm_store(0, 0, dep_all0, hb0, None,
                      eng_ring([nc.scalar, nc.vector]))
            transposes(b1_nats, (0,), hTs1)
            lhq2 = dep_quarter_mm(hTs0, 2)
            dep_quarter_scores(lhq1, 1, dep_all0)
            asm_store(0, 1, dep_all0, hb0, None,
                      eng_ring([nc.gpsimd, nc.scalar, nc.vector, nc.gpsimd]))
            transposes(b1_nats, (1,), hTs1)
            lhq3 = dep_quarter_mm(hTs0, 3)
            dep_quarter_scores(lhq2, 2, dep_all0)
            asm_store(0, 2, dep_all0, hb0, None,
                      eng_ring([nc.gpsimd, nc.vector, nc.scalar, nc.gpsimd]))
            dep_quarter_scores(lhq3, 3, dep_all0)
            asm_store(0, 3, dep_all0, hb0, None,
                      eng_ring([nc.gpsimd, nc.scalar, nc.gpsimd, nc.vector]))

            # ================= batch 1 (last) =================
            lh_head = head_mlp(hTs1, (0,))
            hb1 = {c: hbc_p.tile([P, L], f32, name="hb") for c in range(2)}
            head_bc_phase(lh_head, (0,), hb1)

            dep_all1 = depsc_p.tile([P, 4 * NQ], f32, name="dep_all")
            # quarters 0-1: store the j<512 half as soon as scores land
            lhq0 = dep_quarter_mm(hTs1, 0)
            lhq1 = dep_quarter_mm(hTs1, 1)
            dep_quarter_scores(lhq0, 0, dep_all1)
            asm_store(1, 0, dep_all1, hb1, (0,),
                      eng_ring([nc.scalar, nc.vector]))
            dep_quarter_scores(lhq1, 1, dep_all1)
            asm_store(1, 1, dep_all1, hb1, (0,),
                      eng_ring([nc.gpsimd, nc.vector, nc.gpsimd, nc.scalar]))

            # second head half; then backfill j>=512 for quarters 0-1
            head_mlp(hTs1, (1,), tiles=lh_head)
            head_bc_phase(lh_head, (1,), hb1)
            asm_store(1, 0, dep_all1, hb1, (1,),
                      eng_ring([nc.scalar, nc.vector, nc.gpsimd, nc.gpsimd]))
            asm_store(1, 1, dep_all1, hb1, (1,),
                      eng_ring([nc.gpsimd, nc.vector, nc.gpsimd, nc.scalar]))

            # quarters 2-3: full rows, scores pipelined one behind the mm
            lhq2 = dep_quarter_mm(hTs1, 2)
            lhq3 = dep_quarter_mm(hTs1, 3)
            dep_quarter_scores(lhq2, 2, dep_all1)
            asm_store(1, 2, dep_all1, hb1, None,
                      eng_ring([nc.scalar, nc.vector, nc.gpsimd, nc.gpsimd,
                                nc.scalar, nc.vector, nc.scalar, nc.vector]))
            dep_quarter_scores(lhq3, 3, dep_all1)
            asm_store(1, 3, dep_all1, hb1, None,
                      eng_ring([nc.scalar, nc.vector, nc.gpsimd, nc.scalar,
                                nc.vector, nc.gpsimd, nc.scalar, nc.vector]))

    nc.compile()
    return nc


def _prep_consts(W_dep, b_dep, W_head, b_head, Wc, bc):
    import ml_dtypes

    f = np.float32
    bf = ml_dtypes.bfloat16

    def pad_w(W):
        Wp = np.zeros((D, MLP_PAD), f)
        Wp[:, :MLP] = W
        return Wp.astype(bf)

    def bias_t(bvec):
        bp = np.zeros((MLP_PAD,), f)
        bp[:MLP] = bvec
        bt = bp.reshape(N_MT, P).T  # [P, N_MT]
        out = np.empty((P, 2 * N_MT), f)
        out[:, 0::2] = (1.0 - NEG_SLOPE) * bt
        out[:, 1::2] = bt
        return out

    wc_dep_pad = np.zeros((MLP_PAD, 2), f)
    wc_dep_pad[:MLP] = Wc[:MLP]
    wc_dep_t = wc_dep_pad.reshape(N_MT, P, 2).transpose(1, 0, 2).copy()  # [P,mt,2]

    wc_head_pad = np.zeros((MLP_PAD, 2), f)
    wc_head_pad[:MLP] = Wc[MLP:]
    wh = wc_head_pad.reshape(N_MT, P, 2).transpose(1, 2, 0)  # [P, 2, N_MT]
    wc_head_bc = np.broadcast_to(wh[:, :, :, None], (P, 2, N_MT, P)).copy()

    return {
        "w_dep": pad_w(W_dep),
        "w_head": pad_w(W_head),
        "b_dep_t": bias_t(b_dep),
        "b_head_t": bias_t(b_head),
        "wc_dep_t": wc_dep_t.astype(bf),
        "wc_head_bc": wc_head_bc.astype(bf),
        "bc_bc": np.broadcast_to(bc.astype(f), (P, 2)).copy(),
        "ident": np.eye(P, dtype=f),
    }


def kernel(hidden_state, W_dep, b_dep, W_head, b_head, Wc, bc):
    from concourse.bass_utils import run_bass_kernel_spmd

    hidden_state = np.ascontiguousarray(np.asarray(hidden_state, dtype=np.float32))
    consts = _prep_consts(
        np.asarray(W_dep, np.float32),
        np.asarray(b_dep, np.float32),
        np.asarray(W_head, np.float32),
        np.asarray(b_head, np.float32),
        np.asarray(Wc, np.float32),
        np.asarray(bc, np.float32),
    )

    if "nc" not in _CACHE:
        _CACHE["nc"] = _build_nc()
    nc = _CACHE["nc"]

    in_maps = []
    for k in range(N_CORES):
        m = {"hidden": hidden_state[k * B_PER_CORE : (k + 1) * B_PER_CORE]}
        m.update(consts)
        in_maps.append(m)

    trace = bool(int(os.environ.get("BB_TRACE", "0")))
    if not trace:
        # The NTFF profiling hook (antenv.axon_hooks) is absent in this
        # container; a stray BASS_TRACE=1 would crash the run. Force off.
        os.environ["BASS_NEVER_TRACE"] = "1"
    res = run_bass_kernel_spmd(nc, in_maps, list(range(N_CORES)), trace=trace)
    _CACHE["last_results"] = res
    out = np.concatenate([res.results[k]["out"] for k in range(N_CORES)], axis=0)
    return out


# revision 17
# speedup vs baseline: 1.0398x; 1.0144x over previous
"""Trainium2 Bass kernel for nn_BinaryBiaffine2 (biaffine dependency scorer).

Math (per batch b):
    h_dep  = leaky_relu(hidden @ W_dep  + b_dep)             [L, 500]
    h_head = leaky_relu(hidden @ W_head + b_head)            [L, 500]
    dep    = h_dep  @ Wc[:500]                               [L, 2]
    head   = h_head @ Wc[500:]                               [L, 2]
    out[i, j, c] = dep[i, c] + head[j, c] + bc[c]            [L, L, 2]

Sharding: data-parallel over batch, 2 batches per core on 8 cores.

Per-core strategy (v3):
  - hidden loaded natural ([tok, d]), PE-transposed 128x128 into hT tiles
    [d, tok] stored bf16 (identity is bf16 so transposes run at 1 cyc/row).
  - weights bf16 (halves weight HBM traffic; matmuls full-rate at any N).
  - MLP in [m, tok] layout; leaky(x+b) = relu(0.99x+0.99b) + 0.01*(x+b)
    with the linear term in bf16 so the DVE add runs in 2x mode.
  - head scores broadcast across partitions via matmul with a
    partition-replicated Wc column as stationary (+bc folded in the copy).
  - dep branch runs in token QUARTERS (256 tok) so assembly+stores stream
    at fine granularity; all DMAs (loads+stores) issue from the SP queue
    in readiness order (single shared DMA device in hw).
  - batch 1 (last) computes its head branch in halves: out[i, j in h0]
    tiles store while head-h1 is still being computed, shrinking the
    end-of-kernel store tail.
"""

import os
import sys

if "/opt/trn_rl_repo" not in sys.path:
    sys.path.insert(0, "/opt/trn_rl_repo")

import numpy as np

B, L, D = 16, 1024, 1024
MLP = 500
MLP_PAD = 512
NEG_SLOPE = 0.01
N_CORES = 8
B_PER_CORE = B // N_CORES
P = 128
N_MT = MLP_PAD // P  # 4 m-tiles of 128
N_KO = D // P        # 8 d-slices of 128
N_TSUB = L // P      # 8 token subtiles per batch
NQ = 4               # dep-branch quarters per batch (256 tok each)
QTOK = L // NQ

_CACHE = {}


def _build_nc():
    import concourse.tile as tile
    from concourse import bacc, mybir
    from concourse.bass import ts
    from contextlib import ExitStack

    f32 = mybir.dt.float32
    f32r = mybir.dt.float32r
    bf16 = mybir.dt.bfloat16
    Relu = mybir.ActivationFunctionType.Relu
    Identity = mybir.ActivationFunctionType.Identity
    Add = mybir.AluOpType.add
    Mult = mybir.AluOpType.mult

    nc = bacc.Bacc()

    hidden = nc.dram_tensor("hidden", [B_PER_CORE, L, D], f32r, kind="ExternalInput")
    w_dep_d = nc.dram_tensor("w_dep", [D, MLP_PAD], bf16, kind="ExternalInput")
    w_head_d = nc.dram_tensor("w_head", [D, MLP_PAD], bf16, kind="ExternalInput")
    # bias tiles: columns (2*mt, 2*mt+1) = (0.99*b, b) for m-tile mt
    b_dep_d = nc.dram_tensor("b_dep_t", [P, 2 * N_MT], f32, kind="ExternalInput")
    b_head_d = nc.dram_tensor("b_head_t", [P, 2 * N_MT], f32, kind="ExternalInput")
    wc_dep_d = nc.dram_tensor("wc_dep_t", [P, N_MT, 2], bf16, kind="ExternalInput")
    wc_head_d = nc.dram_tensor("wc_head_bc", [P, 2, N_MT, P], bf16, kind="ExternalInput")
    bc_d = nc.dram_tensor("bc_bc", [P, 2], f32, kind="ExternalInput")
    ident_d = nc.dram_tensor("ident", [P, P], f32r, kind="ExternalInput")
    out_d = nc.dram_tensor("out", [B_PER_CORE, L, L, 2], f32, kind="ExternalOutput")

    with tile.TileContext(nc) as tc:
        with ExitStack() as ctx:
            const = ctx.enter_context(tc.tile_pool(name="const", bufs=1))
            hnat_p = ctx.enter_context(tc.tile_pool(name="hnat", bufs=4))
            hT_p = ctx.enter_context(tc.tile_pool(name="hT", bufs=24))
            lh_p = ctx.enter_context(tc.tile_pool(name="lh", bufs=5))
            lhd_p = ctx.enter_context(tc.tile_pool(name="lhd", bufs=8))
            tmp_p = ctx.enter_context(tc.tile_pool(name="tmp", bufs=3))
            depsc_p = ctx.enter_context(tc.tile_pool(name="depsc", bufs=2))
            hbc_p = ctx.enter_context(tc.tile_pool(name="hbc", bufs=4))
            outh_p = ctx.enter_context(tc.tile_pool(name="outh", bufs=10))
            tr_ps = ctx.enter_context(tc.tile_pool(name="trps", bufs=4, space="PSUM"))
            h_ps = ctx.enter_context(tc.tile_pool(name="hps", bufs=4, space="PSUM"))

            # ---- constants: tiny ones first, then batch-0 hidden, then
            # weights; all on the SP (sync) HWDGE queue which carries every
            # DMA so the shared DMA device runs them in readiness order.
            ident_sb = const.tile([P, P], f32r)
            nc.sync.dma_start(ident_sb, ident_d[:, :])

            hid_r = hidden[:, :, :]

            def load_batch(b, fine):
                h_nats = []
                for tp in range(N_TSUB // 2):
                    h_nat = hnat_p.tile([P, 2, D], f32r, name="h_nat")
                    if fine:
                        for s in range(2):
                            nc.sync.dma_start(
                                h_nat[:, s],
                                hid_r[b, ts(2 * tp + s, P), :],
                            )
                    else:
                        nc.sync.dma_start(
                            h_nat,
                            hid_r[b, ts(tp, 2 * P), :].rearrange(
                                "(s p) d -> p s d", p=P
                            ),
                        )
                    h_nats.append(h_nat)
                return h_nats

            # batch-0 half 0 tokens first, then w_head (so the head MLP can
            # start on half 0 while half 1 + w_dep are still loading)
            w_sb = {}
            loaded = []

            def load_b0_half(half):
                for tp in range(2 * half, 2 * half + 2):
                    h_nat = hnat_p.tile([P, 2, D], f32r, name="h_nat")
                    for s in range(2):
                        nc.sync.dma_start(
                            h_nat[:, s], hid_r[0, ts(2 * tp + s, P), :]
                        )
                    loaded.append(h_nat)

            load_b0_half(0)
            b_sb = {}
            w_head_sb = const.tile([P, N_KO, MLP_PAD], bf16)
            nc.sync.dma_start(
                w_head_sb, w_head_d[:, :].rearrange("(ko p) m -> p ko m", p=P)
            )
            b_head_sb = const.tile([P, 2 * N_MT], f32)
            nc.sync.dma_start(b_head_sb, b_head_d[:, :])
            load_b0_half(1)
            w_dep_sb = const.tile([P, N_KO, MLP_PAD], bf16)
            nc.sync.dma_start(
                w_dep_sb, w_dep_d[:, :].rearrange("(ko p) m -> p ko m", p=P)
            )
            w_sb["dep"], w_sb["head"] = w_dep_sb, w_head_sb
            b_dep_sb = const.tile([P, 2 * N_MT], f32)
            nc.sync.dma_start(b_dep_sb, b_dep_d[:, :])
            b_sb["dep"], b_sb["head"] = b_dep_sb, b_head_sb
            wc_dep_sb = const.tile([P, N_MT, 2], bf16)
            nc.sync.dma_start(wc_dep_sb, wc_dep_d[:, :, :])
            bc_sb = const.tile([P, 2], f32)
            nc.sync.dma_start(bc_sb, bc_d[:, :])
            wc_head_sb = const.tile([P, 2, N_MT, P], bf16)
            nc.sync.dma_start(wc_head_sb, wc_head_d[:, :, :, :])

            # rotating engine assignment for elementwise output/copy units
            def eng_ring(seq):
                i = [0]

                def nxt():
                    e = seq[i[0] % len(seq)]
                    i[0] += 1
                    return e

                return nxt

            def transposes(h_nats, halves=(0, 1), hTs=None):
                """PE-transpose a batch into hT tiles [d=128, tok=512] bf16."""
                if hTs is None:
                    hTs = {}
                cp = eng_ring([nc.vector, nc.vector, nc.scalar])
                for half in halves:
                    for ko in range(N_KO):
                        ptr = tr_ps.tile([P, 512], f32r, name="ptr")
                        for q in range(4):
                            tsub = half * 4 + q
                            nc.tensor.matmul(
                                ptr[:, ts(q, P)],
                                lhsT=h_nats[tsub // 2][:, tsub % 2, ts(ko, P)],
                                rhs=ident_sb,
                                is_transpose=True,
                                start=True,
                                stop=True,
                            )
                        hT = hT_p.tile([P, 512], bf16, name="hT")
                        e = cp()
                        if e is nc.scalar:
                            e.activation(hT, ptr, Identity)
                        else:
                            e.tensor_copy(hT, ptr)
                        hTs[half, ko] = hT
                return hTs

            def leaky(dst, ps, br, mt):
                """dst(bf16) = leaky_relu(ps + b) via relu(0.99x+0.99b) +
                0.01(x+b); the bf16 add runs in DVE 2x mode."""
                lt = tmp_p.tile(list(dst.shape), bf16, name="lt")
                nc.scalar.activation(
                    dst, ps, Relu,
                    bias=b_sb[br][:, 2 * mt : 2 * mt + 1],
                    scale=1.0 - NEG_SLOPE,
                )
                nc.vector.tensor_scalar(
                    lt, ps,
                    b_sb[br][:, 2 * mt + 1 : 2 * mt + 2], NEG_SLOPE,
                    Add, Mult,
                )
                nc.vector.tensor_add(dst, dst, lt)

            def head_mlp(hTs, halves, tiles=None):
                """Head branch for the given token halves -> lh tiles
                [m=128, L] bf16 keyed by mt (columns half*512.. filled).
                Emits all of half A before half B so the PE can run as soon
                as half A's hT tiles exist."""
                if tiles is None:
                    tiles = {mt: lh_p.tile([P, L], bf16, name="lh")
                             for mt in range(N_MT)}
                for half in halves:
                    for mt in range(N_MT):
                        ps = h_ps.tile([P, 512], f32, name="hps")
                        for ko in range(N_KO):
                            nc.tensor.matmul(
                                ps,
                                lhsT=w_sb["head"][:, ko, ts(mt, P)],
                                rhs=hTs[half, ko],
                                start=(ko == 0),
                                stop=(ko == N_KO - 1),
                            )
                        leaky(tiles[mt][:, ts(half, 512)], ps, "head", mt)
                return tiles

            def head_bc_phase(lh_tiles, halves, hb_tiles):
                """Head scores, partition-broadcast, +bc folded; writes the
                [:, half*512:...] columns of hb_tiles[c]."""
                for half in halves:
                    for c in range(2):
                        pbc = h_ps.tile([P, 512], f32, name="hps")
                        for mt in range(N_MT):
                            nc.tensor.matmul(
                                pbc,
                                lhsT=wc_head_sb[:, c, mt, :],
                                rhs=lh_tiles[mt][:, ts(half, 512)],
                                start=(mt == 0),
                                stop=(mt == N_MT - 1),
                            )
                        nc.scalar.activation(
                            hb_tiles[c][:, ts(half, 512)],
                            pbc,
                            Identity,
                            bias=bc_sb[:, c : c + 1],
                        )

            def dep_quarter_mm(hTs, q):
                """Dep-branch MLP for token quarter q -> lh tiles [m, 256]."""
                half, qc = q // 2, q % 2
                tiles = {}
                for mt in range(N_MT):
                    psd = h_ps.tile([P, QTOK], f32, name="hps",
                                    padded_shape=[P, 512])
                    for ko in range(N_KO):
                        nc.tensor.matmul(
                            psd,
                            lhsT=w_sb["dep"][:, ko, ts(mt, P)],
                            rhs=hTs[half, ko][:, ts(qc, QTOK)],
                            start=(ko == 0),
                            stop=(ko == N_KO - 1),
                        )
                    lh = lhd_p.tile([P, QTOK], bf16, name="lhd")
                    leaky(lh, psd, "dep", mt)
                    tiles[mt] = lh
                return tiles

            def dep_scores_tile(lhq, q, t, dep_all):
                """Scores for i-tile t of quarter q, computed directly in
                [tok, 2] layout (lh chunk stationary, wc moving) -> two
                per-token scalar columns of dep_all."""
                psq = tr_ps.tile([P, 2], f32, name="ptr",
                                 padded_shape=[P, 512])
                for mt in range(N_MT):
                    nc.tensor.matmul(
                        psq,
                        lhsT=lhq[mt][:, ts(t, P)],
                        rhs=wc_dep_sb[:, mt, :],
                        start=(mt == 0),
                        stop=(mt == N_MT - 1),
                    )
                col = 4 * q + 2 * t
                nc.vector.tensor_copy(dep_all[:, col : col + 2], psq)

            def dep_quarter_scores(lhq, q, dep_all):
                for t in range(2):
                    dep_scores_tile(lhq, q, t, dep_all)

            def asm_store(b, q, dep_all, hb_tiles, jhalves, eng):
                """Assemble + store out tiles for the 2 i-tiles of quarter q,
                covering the given j-halves (None = both)."""
                if jhalves is None:
                    jhalves = (0, 1)
                for t in range(2):
                    tsub = 2 * q + t
                    d0 = dep_all[:, 4 * q + 2 * t : 4 * q + 2 * t + 1]
                    d1 = dep_all[:, 4 * q + 2 * t + 1 : 4 * q + 2 * t + 2]
                    for jh in jhalves:
                        ot = outh_p.tile([P, 512, 2], f32, name="oth")
                        for c, dap in ((0, d0), (1, d1)):
                            e = eng()
                            src = hb_tiles[c][:, ts(jh, 512)]
                            if e is nc.scalar:
                                e.activation(ot[:, :, c], src, Identity,
                                             bias=dap)
                            else:
                                e.tensor_scalar(ot[:, :, c], src, dap,
                                                None, Add)
                        nc.sync.dma_start(
                            out_d[b, ts(tsub, P), ts(jh, 512)], ot
                        )

            # ================= batch 0 =================
            # PE order: T0h0, H0h0, T0h1, H0h1, bc -- the head branch (and
            # with it the first stores) starts as soon as half-0 tokens and
            # w_head are resident; transposes of later data fill DMA waits.
            hTs0 = {}
            transposes(loaded, (0,), hTs0)
            lh_head = head_mlp(hTs0, (0,))
            transposes(loaded, (1,), hTs0)
            b1_nats = load_batch(1, fine=False)
            head_mlp(hTs0, (1,), tiles=lh_head)
            hb0 = {c: hbc_p.tile([P, L], f32, name="hb") for c in range(2)}
            head_bc_phase(lh_head, (0, 1), hb0)

            dep_all0 = depsc_p.tile([P, 4 * NQ], f32, name="dep_all")
            hTs1 = {}
            lhq0 = dep_quarter_mm(hTs0, 0)
            lhq1 = dep_quarter_mm(hTs0, 1)
            dep_quarter_scores(lhq0, 0, dep_all0)
            asm_store(0, 0, dep_all0, hb0, None,
                      eng_ring([nc.scalar, nc.vector]))
            transposes(b1_nats, (0,), hTs1)
            lhq2 = dep_quarter_mm(hTs0, 2)
            dep_quarter_scores(lhq1, 1, dep_all0)
            asm_store(0, 1, dep_all0, hb0, None,
                      eng_ring([nc.gpsimd, nc.scalar, nc.vector, nc.gpsimd]))
            transposes(b1_nats, (1,), hTs1)
            lhq3 = dep_quarter_mm(hTs0, 3)
            dep_quarter_scores(lhq2, 2, dep_all0)
            asm_store(0, 2, dep_all0, hb0, None,
                      eng_ring([nc.gpsimd, nc.vector, nc.scalar, nc.gpsimd]))
            dep_quarter_scores(lhq3, 3, dep_all0)
            asm_store(0, 3, dep_all0, hb0, None,
                      eng_ring([nc.gpsimd, nc.scalar, nc.gpsimd, nc.vector]))

            # ================= batch 1 (last) =================
            lh_head = head_mlp(hTs1, (0,))
            hb1 = {c: hbc_p.tile([P, L], f32, name="hb") for c in range(2)}
            head_bc_phase(lh_head, (0,), hb1)

            dep_all1 = depsc_p.tile([P, 4 * NQ], f32, name="dep_all")
            # quarters 0-1: store the j<512 half as soon as scores land
            lhq0 = dep_quarter_mm(hTs1, 0)
            lhq1 = dep_quarter_mm(hTs1, 1)
            dep_quarter_scores(lhq0, 0, dep_all1)
            asm_store(1, 0, dep_all1, hb1, (0,),
                      eng_ring([nc.scalar, nc.vector]))
            dep_quarter_scores(lhq1, 1, dep_all1)
            asm_store(1, 1, dep_all1, hb1, (0,),
                      eng_ring([nc.gpsimd, nc.vector, nc.gpsimd, nc.scalar]))

            # second head half; then backfill j>=512 for quarters 0-1
            head_mlp(hTs1, (1,), tiles=lh_head)
            head_bc_phase(lh_head, (1,), hb1)
            asm_store(1, 0, dep_all1, hb1, (1,),
                      eng_ring([nc.scalar, nc.vector, nc.gpsimd, nc.gpsimd]))
            asm_store(1, 1, dep_all1, hb1, (1,),
                      eng_ring([nc.gpsimd, nc.vector, nc.gpsimd, nc.scalar]))

            # quarters 2-3: full rows, scores pipelined one behind the mm
            lhq2 = dep_quarter_mm(hTs1, 2)
            lhq3 = dep_quarter_mm(hTs1, 3)
            dep_quarter_scores(lhq2, 2, dep_all1)
            asm_store(1, 2, dep_all1, hb1, None,
                      eng_ring([nc.scalar, nc.vector, nc.gpsimd, nc.gpsimd,
                                nc.scalar, nc.vector, nc.scalar, nc.vector]))
            dep_quarter_scores(lhq3, 3, dep_all1)
            asm_store(1, 3, dep_all1, hb1, None,
                      eng_ring([nc.scalar, nc.vector, nc.gpsimd, nc.scalar,
                                nc.vector, nc.gpsimd, nc.scalar, nc.vector]))

    nc.compile()
    return nc


def _prep_consts(W_dep, b_dep, W_head, b_head, Wc, bc):
    import ml_dtypes

    f = np.float32
    bf = ml_dtypes.bfloat16

    def pad_w(W):
        Wp = np.zeros((D, MLP_PAD), f)
        Wp[:, :MLP] = W
        return Wp.astype(bf)

    def bias_t(bvec):
        bp = np.zeros((MLP_PAD,), f)
        bp[:MLP] = bvec
        bt = bp.reshape(N_MT, P).T  # [P, N_MT]
        out = np.empty((P, 2 * N_MT), f)
        out[:, 0::2] = (1.0 - NEG_SLOPE) * bt
        out[:, 1::2] = bt
        return out

    wc_dep_pad = np.zeros((MLP_PAD, 2), f)
    wc_dep_pad[:MLP] = Wc[:MLP]
    wc_dep_t = wc_dep_pad.reshape(N_MT, P, 2).transpose(1, 0, 2).copy()  # [P,mt,2]

    wc_head_pad = np.zeros((MLP_PAD, 2), f)
    wc_head_pad[:MLP] = Wc[MLP:]
    wh = wc_head_pad.reshape(N_MT, P, 2).transpose(1, 2, 0)  # [P, 2, N_MT]
    wc_head_bc = np.broadcast_to(wh[:, :, :, None], (P, 2, N_MT, P)).copy()

    return {
        "w_dep": pad_w(W_dep),
        "w_head": pad_w(W_head),
        "b_dep_t": bias_t(b_dep),
        "b_head_t": bias_t(b_head),
        "wc_dep_t": wc_dep_t.astype(bf),
        "wc_head_bc": wc_head_bc.astype(bf),
        "bc_bc": np.broadcast_to(bc.astype(f), (P, 2)).copy(),
        "ident": np.eye(P, dtype=f),
    }


def kernel(hidden_state, W_dep, b_dep, W_head, b_head, Wc, bc):
    from concourse.bass_utils import run_bass_kernel_spmd

    hidden_state = np.ascontiguousarray(np.asarray(hidden_state, dtype=np.float32))
    consts = _prep_consts(
        np.asarray(W_dep, np.float32),
        np.asarray(b_dep, np.float32),
        np.asarray(W_head, np.float32),
        np.asarray(b_head, np.float32),
        np.asarray(Wc, np.float32),
        np.asarray(bc, np.float32),
    )

    if "nc" not in _CACHE:
        _CACHE["nc"] = _build_nc()
    nc = _CACHE["nc"]

    in_maps = []
    for k in range(N_CORES):
        m = {"hidden": hidden_state[k * B_PER_CORE : (k + 1) * B_PER_CORE]}
        m.update(consts)
        in_maps.append(m)

    trace = bool(int(os.environ.get("BB_TRACE", "0")))
    if not trace:
        # The NTFF profiling hook (antenv.axon_hooks) is absent in this
        # container; a stray BASS_TRACE=1 would crash the run. Force off.
        os.environ["BASS_NEVER_TRACE"] = "1"
    res = run_bass_kernel_spmd(nc, in_maps, list(range(N_CORES)), trace=trace)
    _CACHE["last_results"] = res
    out = np.concatenate([res.results[k]["out"] for k in range(N_CORES)], axis=0)
    return out
